# revision 33
# baseline (speedup 1.0000x reference)
"""AdaptedAttention (llama-adapter) Trainium2 kernel, 8-core token-data-parallel.

v4: fp8e4m3 DoubleRow for both big GEMMs, always on (measured rel err
2.5e-3 at the harness gate=0.786, vs the 2e-2 pass bar). Per-MM cost on
this hw is streaming-bound at ~227ns per 512-col pass regardless of
dtype, so DoubleRow's 2 contraction-tiles per pass is a true 2x; the
kernel runs at the MM-count floor (1024 q + 512 p2 + 96 sc/den passes
~= 370us) plus small head/tail.

Structure:
  - 8192 tokens split 1024/core; weights replicated; no collectives.
  - Adapter K/V and per-head V_h @ Wo_h folded on host in f64. Phase-2
    is probs @ VWO with contraction 2048 (32 heads x 64).
  - Head-paired softmax in one PSUM tile; denominators via
    block-diag-ones matmul; probs = exp(sc - ln den + ln 16) into fp8.
  - j-innermost matmul loops + a post-build LDWEIGHTS dedupe pass (each
    weight tile loads once, streams both 512-token halves).
  - Startup: x tiles stream before weights; no gpsimd DMAs (software
    DGE library load otherwise delays kernel start ~3us); first phase-2
    weight tiles prefetch mid-phase-1 from a dedicated pool.
  - Output stored bf16 (halves store traffic), upcast on host.
"""
import os
import math
import numpy as np

import concourse.bass as bass
import concourse.tile as tile
from concourse import mybir
from concourse.bass_utils import run_bass_kernel_spmd

F32 = mybir.dt.float32
BF16 = mybir.dt.bfloat16
F8 = mybir.dt.float8e4
P = 128

NUM_HEADS = 32
B, S, H, L = 4, 2048, 4096, 64
HD = H // NUM_HEADS            # 128
NC = 8
TOK = (B * S) // NC            # 1024 tokens per core
KT = H // P                    # 32 contraction tiles for q-proj
MT = H // P                    # 32 output feature tiles
NPAIR = NUM_HEADS // 2         # 16 head pairs
NB = 2                         # 512-wide matmul chunks (PSUM bank limit)
NW = TOK // NB
# fp8 build measured at rel-err 2.5e-3 on the harness inputs (gate=0.786),
# ~8x inside the 2e-2 gate, and ~1.2x faster than the bf16 build. Use it
# for any gate; env override kept for A/B testing.
FP8_GATE_MAX = float(os.environ.get("KFP8_GATE", "10.0"))

_PATCHED = False


def dedupe_ldweights(nc):
    """Remove InstLdweights that reload the exact weights already resident
    in the PE array (same physical AP + perf_mode, no intervening weight
    change, no sync side effects). The following matmuls then reuse the
    loaded stationary operand; verified correct on hw (microbench)."""
    removed = 0
    skippable = ("InstMatmult", "InstNoOp", "InstEventSemaphore")
    for f in nc.m.functions:
        for b in f.blocks:
            insts = b.instructions
            last_sig = None
            to_remove = []
            for i in insts:
                tn = type(i).__name__
                if tn == "InstLdweights":
                    si = i.sync_info
                    clean = si is None or (not si.on_wait and not si.on_update)
                    sig = (repr(i.ins[0]), str(i.perf_mode), bool(i.is_transpose))
                    if sig == last_sig and clean:
                        to_remove.append(i)
                        continue
                    last_sig = sig
                elif tn in skippable:
                    pass
                elif getattr(i, "engine", None) == mybir.EngineType.PE:
                    last_sig = None
            for i in to_remove:
                insts.remove(i)
            removed += len(to_remove)
    return removed


def _patch_tile():
    """TRN2 instructions have one hw wait slot; walrus rejects multi-wait
    matmuls and the kernel-tail drain. Hoist extra waits onto NoOps."""
    global _PATCHED
    if _PATCHED:
        return
    _PATCHED = True
    import concourse.tile as _tile
    from concourse.vector_clock import ScopedClock

    _orig_commit = _tile.TileContext._commit_instruction

    def _patched_commit(self, inst, lazy_reg_writes=True):
        si = getattr(inst, "sync_info", None)
        if (
            si is not None
            and si.on_wait
            and len(si.on_wait) > 1
            and inst.engine != mybir.EngineType.Unassigned
            and not isinstance(inst, mybir.InstNoOp)
        ):
            waits = list(si.on_wait)
            for w in waits[:-1]:
                nop = mybir.InstNoOp(
                    name=self.nc.get_next_instruction_name(),
                    ins=[], outs=[], bass_nofuse=True,
                )
                nop.engine = inst.engine
                nop.sync_info = mybir.SyncInfo(on_wait=[w], on_update=[])
                _orig_commit(self, nop, lazy_reg_writes=False)
            inst.sync_info = mybir.SyncInfo(
                on_wait=waits[-1:], on_update=list(si.on_update or [])
            )
        return _orig_commit(self, inst, lazy_reg_writes=lazy_reg_writes)

    def _patched_drain_and_barrier(self, tick_clock, wait_clock):
        nc = self.nc
        carrier = nc.sync.nop(nofuse=True)
        wait_clock.add_sem_waits(
            carrier.ins, ScopedClock({None: tick_clock.global_clock})
        )
        si = carrier.ins.sync_info
        waits = list(si.on_wait) if si and si.on_wait else []
        if len(waits) > 1:
            carrier.ins.sync_info = mybir.SyncInfo(
                on_wait=waits[:1], on_update=list(si.on_update or [])
            )
            for w in waits[1:]:
                extra = nc.sync.nop(nofuse=True)
                extra.ins.sync_info = mybir.SyncInfo(on_wait=[w], on_update=[])
        nc.sync.drain()
        nc.all_engine_barrier()
        assert self.sems is not None
        popped = nc._tile_sem_poison_stack.pop()
        assert popped is self._sem_poison
        nc.clear_and_free_semaphores(list(self.sems.allocated().values()))
        nc.all_engine_barrier()

    _tile.TileContext._commit_instruction = _patched_commit
    _tile.TileContext._drain_and_barrier = _patched_drain_and_barrier


def build_fp8():
    _patch_tile()
    from contextlib import ExitStack

    nc = bass.Bass(target_bir_lowering=False)

    xq8 = nc.declare_dram_parameter("xq8", [KT // 2, P, 2, TOK], F8, isOutput=False)
    qw8 = nc.declare_dram_parameter("qw8", [MT, 4, P, 4, 2, P], F8, isOutput=False)
    vw8 = nc.declare_dram_parameter("vw8", [MT, 4, P, 2, 2, P], F8, isOutput=False)
    katt = nc.declare_dram_parameter("katt", [P, NUM_HEADS * L], BF16, isOutput=False)
    blkones = nc.declare_dram_parameter("blkones", [P, P], BF16, isOutput=False)
    cosT = nc.declare_dram_parameter("cosT", [HD, TOK], BF16, isOutput=False)
    srotT = nc.declare_dram_parameter("srotT", [HD, TOK], BF16, isOutput=False)
    qb2 = nc.declare_dram_parameter("qb2", [P, MT], F32, isOutput=False)
    qbr = nc.declare_dram_parameter("qbr", [P, MT], F32, isOutput=False)
    baseT = nc.declare_dram_parameter("baseT", [H, TOK], BF16, isOutput=False)
    dsc = nc.declare_dram_parameter("dsc", [P, 1], F32, isOutput=False)
    outT = nc.declare_dram_parameter("outT", [H, TOK], BF16, isOutput=True)

    LN16 = float(np.log(16.0))

    with tile.TileContext(nc) as tc:
        es = ExitStack()
        persist = es.enter_context(tc.tile_pool(name="persist", bufs=1))

        ph_es = ExitStack()
        psQ = ph_es.enter_context(tc.tile_pool(name="psQ", bufs=4, space="PSUM"))
        psS = ph_es.enter_context(tc.tile_pool(name="psS", bufs=2, space="PSUM"))
        wpool = ph_es.enter_context(tc.tile_pool(name="wpool", bufs=16))
        probs_pool = ph_es.enter_context(tc.tile_pool(name="probs", bufs=1))
        # dedicated pool for the first phase-2 weight tiles (prefetched
        # mid-phase-1); must be created before the xt-scope pools so pool
        # releases stay LIFO
        vpool = ph_es.enter_context(tc.tile_pool(name="vpool", bufs=12))
        xt_es = ExitStack()
        xt_pool = xt_es.enter_context(tc.tile_pool(name="xt", bufs=1))
        rpool = xt_es.enter_context(tc.tile_pool(name="rpool", bufs=2))
        tpool = xt_es.enter_context(tc.tile_pool(name="tpool", bufs=2))

        # ---- urgent streams, interleaved by first-use: the first matmuls
        # need xq d0-d3 + the first head's 4 weight tiles, so those lead;
        # the rest follow. gpsimd DMAs avoided everywhere: software DGE
        # pulls in a library load that delays kernel start by ~3us.
        xq_tiles = [None] * (KT // 2)
        qw_pref = {}

        def _load_xq(d, eng):
            t = xt_pool.tile([P, 2, TOK], F8, name=f"xq_{d}", tag=f"xq_{d}")
            eng.dma_start(out=t[:], in_=xq8[d])
            xq_tiles[d] = t

        def _load_qw(h, c, eng):
            wb = wpool.tile([P, 4, 2, P], F8, name=f"qw_{h}_{c}", tag="wq")
            eng.dma_start(out=wb[:], in_=qw8[h, c])
            qw_pref[(h, c)] = wb

        # heads are processed in rotated order HSEQ (see below): first two
        # processed heads are 4 and 5
        _load_xq(0, nc.sync)
        _load_xq(1, nc.scalar)
        _load_qw(4, 0, nc.sync)
        _load_qw(4, 1, nc.scalar)
        _load_xq(2, nc.sync)
        _load_xq(3, nc.scalar)
        _load_qw(4, 2, nc.sync)
        _load_qw(4, 3, nc.scalar)
        for d in range(4, KT // 2):
            _load_xq(d, (nc.sync, nc.scalar)[d % 2])
        for c in range(4):
            _load_qw(5, c, (nc.sync, nc.scalar)[c % 2])

        cos_sb = persist.tile([HD, TOK], BF16, name="cos_sb")
        nc.sync.dma_start(out=cos_sb[:], in_=cosT[:])
        srot_sb = persist.tile([HD, TOK], BF16, name="srot_sb")
        nc.scalar.dma_start(out=srot_sb[:], in_=srotT[:])
        katt_sb = persist.tile([P, NUM_HEADS * L], BF16, name="katt_sb")
        nc.sync.dma_start(out=katt_sb[:], in_=katt[:])
        blk_sb = persist.tile([P, P], BF16, name="blk_sb")
        nc.sync.dma_start(out=blk_sb[:], in_=blkones[:])
        qb2_sb = persist.tile([P, MT], F32, name="qb2_sb")
        nc.scalar.dma_start(out=qb2_sb[:], in_=qb2[:])
        qbr_sb = persist.tile([P, MT], F32, name="qbr_sb")
        nc.scalar.dma_start(out=qbr_sb[:], in_=qbr[:])
        dsc_sb = persist.tile([P, 1], F32, name="dsc_sb")
        nc.sync.dma_start(out=dsc_sb[:], in_=dsc[:])
        ln16_sb = persist.tile([P, 1], F32, name="ln16_sb")
        nc.vector.memset(ln16_sb[:], LN16)
        s16_sb = persist.tile([P, 1], F32, name="s16_sb")
        nc.vector.memset(s16_sb[:], 16.0)
        warm_sb = persist.tile([P, 64], BF16, name="warm_sb")
        nc.vector.memset(warm_sb[:], 0.0)

        probs_quads = [
            probs_pool.tile([P, 2, TOK], F8, name=f"probs_{q}", tag=f"probs_{q}")
            for q in range(NPAIR // 2)
        ]

        sc_pair = [None]
        expt_tiles = {}
        den_tiles = {}

        def emit_q(h, pqs=None):
            # per-j [P, NW] psum tiles on a 4-deep ring: each half frees as
            # soon as its own drain copy runs, so the next q block never
            # waits on the slower half's reader
            if pqs is None:
                pqs = [psQ.tile([P, NW], F32, name=f"pq_{h}_{j}", tag="psQ",
                                bufs=4) for j in range(NB)]
            wbs = []
            for c in range(4):
                if (h, c) in qw_pref:
                    wb = qw_pref[(h, c)]
                else:
                    wb = wpool.tile([P, 4, 2, P], F8, name=f"qw_{h}_{c}", tag="wq")
                    eng = (nc.sync, nc.scalar)[(h * 4 + c) % 2]
                    eng.dma_start(out=wb[:], in_=qw8[h, c])
                wbs.append(wb)
            # j innermost: both 512-token halves stream against one weight
            # load (the dedupe pass drops the second LDWEIGHTS)
            for c in range(4):
                for dd in range(4):
                    d = c * 4 + dd
                    for j in range(NB):
                        nc.tensor.matmul(
                            out=pqs[j][:],
                            lhsT=wbs[c][:, dd, :, :],
                            rhs=xq_tiles[d][:, :, j * NW:(j + 1) * NW],
                            start=(d == 0), stop=(d == KT // 2 - 1),
                            perf_mode=mybir.MatmulPerfMode.DoubleRow,
                        )
            return pqs

        def emit_rope(h, pqs):
            q_sb = rpool.tile([P, TOK], BF16, name=f"q_sb_{h}", tag="q_sb")
            # split the PSUM drain across two engines so each half's buffer
            # frees independently
            nc.scalar.copy(out=q_sb[:, :NW], in_=pqs[0][:])
            nc.vector.tensor_copy(out=q_sb[:, NW:], in_=pqs[1][:])
            rot = rpool.tile([P, TOK], BF16, name=f"rot_{h}", tag="rot")
            nc.scalar.dma_start(out=rot[0:64, :], in_=q_sb[64:128, :])
            nc.sync.dma_start(out=rot[64:128, :], in_=q_sb[0:64, :])
            t2 = rpool.tile([P, TOK], BF16, name=f"t2_{h}", tag="t2")
            nc.vector.scalar_tensor_tensor(
                out=t2[:], in0=q_sb[:], scalar=qb2_sb[:, h:h + 1],
                in1=cos_sb[:], op0=mybir.AluOpType.add,
                op1=mybir.AluOpType.mult)
            t1 = rpool.tile([P, TOK], BF16, name=f"t1_{h}", tag="t1")
            nc.vector.scalar_tensor_tensor(
                out=t1[:], in0=rot[:], scalar=qbr_sb[:, h:h + 1],
                in1=srot_sb[:], op0=mybir.AluOpType.add,
                op1=mybir.AluOpType.mult)
            qr = rpool.tile([P, TOK], BF16, name=f"qr_{h}", tag="qr", bufs=3)
            nc.vector.tensor_add(out=qr[:], in0=t2[:], in1=t1[:])
            return qr

        qr_even = [None]

        def emit_sc(h, qr):
            # defer the even head; emit both heads' score matmuls together
            # so the two 64-partition outputs run CONCURRENTLY in separate
            # PE column groups (tile_position derives from out partition
            # offset) — 4 MMs in ~2 MM times instead of 4
            hh = h % 2
            p = h // 2
            if hh == 0:
                qr_even[0] = qr
                return
            sc = psS.tile([P, TOK], F32, name=f"sc_{p}", tag="psS")
            sc_pair[0] = sc
            for j in range(NB):
                for hh2, qrx in ((0, qr_even[0]), (1, qr)):
                    hx = 2 * p + hh2
                    nc.tensor.matmul(
                        out=sc[hh2 * 64:(hh2 + 1) * 64, j * NW:(j + 1) * NW],
                        lhsT=katt_sb[:, hx * L:(hx + 1) * L],
                        rhs=qrx[:, j * NW:(j + 1) * NW],
                        start=True, stop=True,
                    )
            expt = tpool.tile([P, TOK], BF16, name=f"expt_{p}",
                              tag="expt", bufs=2)
            nc.scalar.activation(expt[:], sc[:],
                                 mybir.ActivationFunctionType.Exp)
            expt_tiles[p] = (expt, sc)

        def emit_den(p, fast_tail=False):
            expt, scp = expt_tiles.pop(p)
            dens = [psQ.tile([P, NW], F32, name=f"den_{p}_{j}", tag="psQ",
                             bufs=4) for j in range(NB)]
            for j in range(NB):
                nc.tensor.matmul(
                    out=dens[j][:],
                    lhsT=blk_sb[:],
                    rhs=expt[:, j * NW:(j + 1) * NW],
                    start=True, stop=True,
                )
            # NOTE: a vector.reciprocal softmax tail was tried here and is
            # ~3.3us per 512-col half on DVE — far slower than this chain.
            lnden = tpool.tile([P, TOK], F32, name=f"lnden_{p}",
                               tag="lnden", bufs=2)
            d2 = tpool.tile([P, TOK], F32, name=f"d2_{p}", tag="d2", bufs=2)
            if fast_tail:
                # last pair: fully j-split ln->sub->exp chain so the j0
                # half of the quad is ready ~1.2us earlier for phase 2
                for j in range(NB):
                    sl = slice(j * NW, (j + 1) * NW)
                    nc.scalar.activation(lnden[:, sl], dens[j][:],
                                         mybir.ActivationFunctionType.Ln)
                    nc.vector.tensor_sub(out=d2[:, sl], in0=scp[:, sl],
                                         in1=lnden[:, sl])
                    nc.scalar.activation(
                        probs_quads[p // 2][:, p % 2, sl], d2[:, sl],
                        mybir.ActivationFunctionType.Exp,
                        bias=ln16_sb[:, 0:1])
                return
            for j in range(NB):
                nc.scalar.activation(lnden[:, j * NW:(j + 1) * NW],
                                     dens[j][:],
                                     mybir.ActivationFunctionType.Ln)
            nc.vector.tensor_sub(out=d2[:], in0=scp[:], in1=lnden[:])
            nc.scalar.activation(
                probs_quads[p // 2][:, p % 2, :], d2[:],
                mybir.ActivationFunctionType.Exp, bias=ln16_sb[:, 0:1])

        # Heads processed in rotated order: heads 0..3 (probs quad 0) come
        # LAST, and phase 2 contracts quad 0 last, so the final heads'
        # softmax chain (a ~5us scalar/vector serial tail) hides behind
        # phase-2 matmuls on quads 1..7.
        HSEQ = list(range(4, NUM_HEADS)) + [0, 1, 2, 3]
        PSEQ = [h // 2 for h in HSEQ[::2]]  # pair completion order

        # first phase-2 weight tiles prefetch mid-phase-1 from vpool so
        # their DMAs are not gated on the q-weight ring freeing (that
        # wait was a 4.4us PE stall at the phase boundary)
        vwo_pref = {}
        m0_pos = [None]
        m0_wbs = [None]

        # HAM warm-up: the PE sits idle ~13us waiting for the first input
        # DMAs, during which the clock gate stays at 4/8 (1.2 GHz) and the
        # first ~3.4us of real matmuls run at half speed. Burn ~3.4us of
        # zero matmuls on an already-memset tile to trip the activity
        # window; they overwrite a corner of head-HSEQ[0]'s psum tile,
        # which the first real accumulation (start=True) clears anyway.
        warm_pqs = [psQ.tile([P, NW], F32, name=f"pq_{HSEQ[0]}_{j}",
                             tag="psQ", bufs=4) for j in range(NB)]
        for _ in range(48):
            nc.tensor.matmul(out=warm_pqs[0][0:64, 0:64],
                             lhsT=warm_sb[:, 0:64], rhs=warm_sb[:, 0:64],
                             start=True, stop=True)

        pq_of = {}
        for step in range(NUM_HEADS):
            pq_of[HSEQ[step]] = emit_q(
                HSEQ[step], pqs=(warm_pqs if step == 0 else None))
            if step == 26:
                for m in range(3):
                    for c in range(4):
                        wb = vpool.tile([P, 2, 2, P], F8,
                                        name=f"vw_{m}_{c}", tag="wv0")
                        eng = (nc.sync, nc.scalar)[(m * 4 + c) % 2]
                        eng.dma_start(out=wb[:], in_=vw8[m, c])
                        vwo_pref[(m, c)] = wb
            if step >= 1:
                h = HSEQ[step - 1]
                qr = emit_rope(h, pq_of.pop(h))
                emit_sc(h, qr)
            if step >= 4 and (step % 2) == 0:
                emit_den(PSEQ[(step - 4) // 2])

        # ---- explicit phase-1 tail, ordered to keep the PE fed while the
        # last pairs' softmax drains on scalar/vector:
        #   rope(3) queued first (vector chain overlaps the PE work below)
        #   den(p0), then m0's quads 1..7 (3.2us of PE work),
        #   sc pair p1, then den(p1) via the vector-only recip path.
        h_last = HSEQ[-1]
        qr_last = emit_rope(h_last, pq_of.pop(h_last))
        emit_den(PSEQ[14])
        pos = [psQ.tile([P, NW], F32, name=f"po_0_{j}", tag="psQ",
                        bufs=4) for j in range(NB)]
        wbs = [vwo_pref[(0, c)] for c in range(4)]
        for q in range(1, NPAIR // 2):
            for j in range(NB):
                nc.tensor.matmul(
                    out=pos[j][:],
                    lhsT=wbs[q // 2][:, q % 2, :, :],
                    rhs=probs_quads[q][:, :, j * NW:(j + 1) * NW],
                    start=(q == 1), stop=False,
                    perf_mode=mybir.MatmulPerfMode.DoubleRow,
                )
        m0_pos[0] = pos
        m0_wbs[0] = wbs
        emit_sc(h_last, qr_last)
        emit_den(PSEQ[15])

        xt_es.close()

        # ---- phase 2: out = probs @ VWO (DR) * dsc + base ----
        p2_es = ExitStack()
        bpool = p2_es.enter_context(tc.tile_pool(name="bpool", bufs=4))

        # quad 0 contracts last (its probs complete last under HSEQ)
        QSEQ = list(range(1, NPAIR // 2)) + [0]
        for m in range(MT):
            if m == 0:
                # quads 1..7 already emitted at the phase boundary
                pos, wbs = m0_pos[0], m0_wbs[0]
                qlist = [0]
            else:
                pos = [psQ.tile([P, NW], F32, name=f"po_{m}_{j}", tag="psQ",
                                bufs=4) for j in range(NB)]
                wbs = []
                for c in range(4):
                    if (m, c) in vwo_pref:
                        wb = vwo_pref[(m, c)]
                    else:
                        wb = wpool.tile([P, 2, 2, P], F8, name=f"vw_{m}_{c}",
                                        tag="wv")
                        eng = (nc.sync, nc.scalar)[(m * 4 + c) % 2]
                        eng.dma_start(out=wb[:], in_=vw8[m, c])
                    wbs.append(wb)
                qlist = QSEQ
            bt = bpool.tile([P, TOK], BF16, name=f"bt_{m}", tag="bt")
            nc.scalar.dma_start(out=bt[:], in_=baseT[m * P:(m + 1) * P, :])
            for q in qlist:
                for j in range(NB):
                    nc.tensor.matmul(
                        out=pos[j][:],
                        lhsT=wbs[q // 2][:, q % 2, :, :],
                        rhs=probs_quads[q][:, :, j * NW:(j + 1) * NW],
                        start=(m != 0 and q == QSEQ[0]), stop=(q == 0),
                        perf_mode=mybir.MatmulPerfMode.DoubleRow,
                    )
            os_ = bpool.tile([P, TOK], BF16, name=f"os_{m}", tag="os")
            for j in range(NB):
                nc.vector.scalar_tensor_tensor(
                    out=os_[:, j * NW:(j + 1) * NW], in0=pos[j][:],
                    scalar=dsc_sb[:, 0:1],
                    in1=bt[:, j * NW:(j + 1) * NW], op0=mybir.AluOpType.mult,
                    op1=mybir.AluOpType.add)
                (nc.sync, nc.scalar)[(2 * m + j) % 2].dma_start(
                    out=outT[m * P:(m + 1) * P, j * NW:(j + 1) * NW],
                    in_=os_[:, j * NW:(j + 1) * NW])

        p2_es.close()
        ph_es.close()
        es.close()
    dedupe_ldweights(nc)
    return nc


def build_bf16():
    _patch_tile()
    from contextlib import ExitStack

    nc = bass.Bass(target_bir_lowering=False)

    xT = nc.declare_dram_parameter("xT", [H, TOK], BF16, isOutput=False)
    qwB = nc.declare_dram_parameter("qwB", [MT, 8, P, 4, P], BF16, isOutput=False)
    vwoB = nc.declare_dram_parameter("vwoB", [MT, 4, P, 4, P], BF16, isOutput=False)
    katt = nc.declare_dram_parameter("katt", [P, NUM_HEADS * L], BF16, isOutput=False)
    blkones = nc.declare_dram_parameter("blkones", [P, P], BF16, isOutput=False)
    cosT = nc.declare_dram_parameter("cosT", [HD, TOK], F32, isOutput=False)
    srotT = nc.declare_dram_parameter("srotT", [HD, TOK], F32, isOutput=False)
    qb2 = nc.declare_dram_parameter("qb2", [P, MT], F32, isOutput=False)
    qbr = nc.declare_dram_parameter("qbr", [P, MT], F32, isOutput=False)
    baseT = nc.declare_dram_parameter("baseT", [H, TOK], F32, isOutput=False)
    outT = nc.declare_dram_parameter("outT", [H, TOK], F32, isOutput=True)

    with tile.TileContext(nc) as tc:
        es = ExitStack()
        persist = es.enter_context(tc.tile_pool(name="persist", bufs=1))

        ph_es = ExitStack()
        psQ = ph_es.enter_context(tc.tile_pool(name="psQ", bufs=2, space="PSUM"))
        psS = ph_es.enter_context(tc.tile_pool(name="psS", bufs=1, space="PSUM"))
        psD = ph_es.enter_context(tc.tile_pool(name="psD", bufs=1, space="PSUM"))
        wpool = ph_es.enter_context(tc.tile_pool(name="wpool", bufs=20))
        probs_pool = ph_es.enter_context(tc.tile_pool(name="probs", bufs=1))
        xt_es = ExitStack()
        xt_pool = xt_es.enter_context(tc.tile_pool(name="xt", bufs=1))
        rpool = xt_es.enter_context(tc.tile_pool(name="rpool", bufs=2))
        tpool = xt_es.enter_context(tc.tile_pool(name="tpool", bufs=2))

        qw_pref = {}
        for h in range(2):
            for kg in range(8):
                wb = wpool.tile([P, 4, P], BF16, name=f"qw_{h}_{kg}", tag="wq")
                eng = (nc.sync, nc.scalar, nc.gpsimd)[kg % 3]
                eng.dma_start(out=wb[:], in_=qwB[h, kg])
                qw_pref[(h, kg)] = wb
        xt_tiles = []
        for k in range(KT):
            t = xt_pool.tile([P, TOK], BF16, name=f"xt_{k}", tag=f"xt_{k}")
            eng = (nc.sync, nc.scalar, nc.gpsimd)[k % 3]
            eng.dma_start(out=t[:], in_=xT[k * P:(k + 1) * P, :])
            xt_tiles.append(t)

        cos_sb = persist.tile([HD, TOK], F32, name="cos_sb")
        nc.gpsimd.dma_start(out=cos_sb[:], in_=cosT[:])
        srot_sb = persist.tile([HD, TOK], F32, name="srot_sb")
        nc.scalar.dma_start(out=srot_sb[:], in_=srotT[:])
        katt_sb = persist.tile([P, NUM_HEADS * L], BF16, name="katt_sb")
        nc.sync.dma_start(out=katt_sb[:], in_=katt[:])
        blk_sb = persist.tile([P, P], BF16, name="blk_sb")
        nc.sync.dma_start(out=blk_sb[:], in_=blkones[:])
        qb2_sb = persist.tile([P, MT], F32, name="qb2_sb")
        nc.gpsimd.dma_start(out=qb2_sb[:], in_=qb2[:])
        qbr_sb = persist.tile([P, MT], F32, name="qbr_sb")
        nc.scalar.dma_start(out=qbr_sb[:], in_=qbr[:])

        probs_tiles = [
            probs_pool.tile([P, TOK], BF16, name=f"probs_{p}", tag=f"probs_{p}")
            for p in range(NPAIR)
        ]

        sc_pair = [None]
        expt_tiles = {}

        def emit_q(h):
            pq = psQ.tile([P, TOK], F32, name=f"pq_{h}", tag="psQ")
            wbs = []
            for kg in range(8):
                if (h, kg) in qw_pref:
                    wb = qw_pref[(h, kg)]
                else:
                    wb = wpool.tile([P, 4, P], BF16, name=f"qw_{h}_{kg}", tag="wq")
                    eng = (nc.sync, nc.scalar, nc.gpsimd)[(h * 8 + kg) % 3]
                    eng.dma_start(out=wb[:], in_=qwB[h, kg])
                wbs.append(wb)
            for j in range(NB):
                for kg in range(8):
                    for kk in range(4):
                        k = kg * 4 + kk
                        nc.tensor.matmul(
                            out=pq[:, j * NW:(j + 1) * NW],
                            lhsT=wbs[kg][:, kk, :],
                            rhs=xt_tiles[k][:, j * NW:(j + 1) * NW],
                            start=(k == 0), stop=(k == KT - 1),
                        )
            return pq

        def emit_rope(h, pq):
            q_sb = rpool.tile([P, TOK], BF16, name=f"q_sb_{h}", tag="q_sb")
            nc.scalar.copy(out=q_sb[:], in_=pq[:])
            rot = rpool.tile([P, TOK], BF16, name=f"rot_{h}", tag="rot")
            nc.scalar.dma_start(out=rot[0:64, :], in_=q_sb[64:128, :])
            nc.sync.dma_start(out=rot[64:128, :], in_=q_sb[0:64, :])
            t2 = rpool.tile([P, TOK], F32, name=f"t2_{h}", tag="t2")
            nc.vector.scalar_tensor_tensor(
                out=t2[:], in0=q_sb[:], scalar=qb2_sb[:, h:h + 1],
                in1=cos_sb[:], op0=mybir.AluOpType.add,
                op1=mybir.AluOpType.mult)
            t1 = rpool.tile([P, TOK], F32, name=f"t1_{h}", tag="t1")
            nc.vector.scalar_tensor_tensor(
                out=t1[:], in0=rot[:], scalar=qbr_sb[:, h:h + 1],
                in1=srot_sb[:], op0=mybir.AluOpType.add,
                op1=mybir.AluOpType.mult)
            qr = rpool.tile([P, TOK], BF16, name=f"qr_{h}", tag="qr", bufs=3)
            nc.vector.tensor_add(out=qr[:], in0=t2[:], in1=t1[:])
            return qr

        def emit_sc(h, qr):
            hh = h % 2
            p = h // 2
            if hh == 0:
                sc_pair[0] = psS.tile([P, TOK], F32, name=f"sc_{p}", tag="psS")
            for j in range(NB):
                nc.tensor.matmul(
                    out=sc_pair[0][hh * 64:(hh + 1) * 64, j * NW:(j + 1) * NW],
                    lhsT=katt_sb[:, h * L:(h + 1) * L],
                    rhs=qr[:, j * NW:(j + 1) * NW],
                    start=True, stop=True,
                )
            if hh == 1:
                expt = tpool.tile([P, TOK], BF16, name=f"expt_{p}",
                                  tag="expt", bufs=2)
                nc.scalar.activation(expt[:], sc_pair[0][:],
                                     mybir.ActivationFunctionType.Exp)
                expt_tiles[p] = expt

        def emit_den(p):
            expt = expt_tiles.pop(p)
            den = psD.tile([P, TOK], F32, name=f"den_{p}", tag="psD")
            for j in range(NB):
                nc.tensor.matmul(
                    out=den[:, j * NW:(j + 1) * NW],
                    lhsT=blk_sb[:],
                    rhs=expt[:, j * NW:(j + 1) * NW],
                    start=True, stop=True,
                )
            recip = tpool.tile([P, TOK], F32, name=f"recip_{p}",
                               tag="recip", bufs=2)
            nc.vector.reciprocal(out=recip[:], in_=den[:])
            nc.vector.tensor_mul(out=probs_tiles[p][:], in0=expt[:],
                                 in1=recip[:])

        pq_of = {}
        for step in range(NUM_HEADS + 2):
            if step < NUM_HEADS:
                pq_of[step] = emit_q(step)
            if step >= 3 and (step % 2) == 1:
                emit_den((step - 3) // 2)
            if 1 <= step <= NUM_HEADS:
                h = step - 1
                qr = emit_rope(h, pq_of.pop(h))
                emit_sc(h, qr)

        vwo_pref = {}
        for m in range(2):
            for g in range(4):
                wb = wpool.tile([P, 4, P], BF16, name=f"vw_{m}_{g}", tag="wq")
                eng = (nc.sync, nc.scalar, nc.gpsimd)[(m * 4 + g) % 3]
                eng.dma_start(out=wb[:], in_=vwoB[m, g])
                vwo_pref[(m, g)] = wb

        xt_es.close()

        p2_es = ExitStack()
        bpool = p2_es.enter_context(tc.tile_pool(name="bpool", bufs=3))

        for m in range(MT):
            po = psQ.tile([P, TOK], F32, name=f"po_{m}", tag="psQ")
            wbs = []
            for g in range(4):
                if (m, g) in vwo_pref:
                    wb = vwo_pref[(m, g)]
                else:
                    wb = wpool.tile([P, 4, P], BF16, name=f"vw_{m}_{g}", tag="wq")
                    eng = (nc.sync, nc.scalar, nc.gpsimd)[(m * 4 + g) % 3]
                    eng.dma_start(out=wb[:], in_=vwoB[m, g])
                wbs.append(wb)
            for j in range(NB):
                for g in range(4):
                    for i in range(4):
                        pr = g * 4 + i
                        nc.tensor.matmul(
                            out=po[:, j * NW:(j + 1) * NW],
                            lhsT=wbs[g][:, i, :],
                            rhs=probs_tiles[pr][:, j * NW:(j + 1) * NW],
                            start=(pr == 0), stop=(pr == NPAIR - 1),
                        )
            bt = bpool.tile([P, TOK], F32, name=f"bt_{m}", tag="bt")
            nc.scalar.dma_start(out=bt[:], in_=baseT[m * P:(m + 1) * P, :])
            os_ = bpool.tile([P, TOK], F32, name=f"os_{m}", tag="os")
            nc.vector.tensor_add(out=os_[:], in0=po[:], in1=bt[:])
            (nc.sync, nc.scalar)[m % 2].dma_start(
                out=outT[m * P:(m + 1) * P, :], in_=os_[:])

        p2_es.close()
        ph_es.close()
        es.close()
    return nc


_NC_CACHE = {}


def _pow2_scale(absmax, target=224.0):
    return 2.0 ** math.floor(math.log2(target / max(absmax, 1e-30)))


def kernel(hidden_states, position_ids, base_output, cos, sin,
           q_w, k_w, v_w, o_w,
           q_scale, k_scale, v_scale, o_scale,
           q_bias, k_bias, v_bias, o_bias,
           adaption_prompt, adaption_gate):
    import ml_dtypes

    hidden_states = np.asarray(hidden_states, dtype=np.float32)
    base_output = np.asarray(base_output, dtype=np.float32)
    pos = np.asarray(position_ids).reshape(-1).astype(np.int64)
    cos = np.asarray(cos, dtype=np.float32)
    sin = np.asarray(sin, dtype=np.float32)

    X = hidden_states.reshape(B * S, H)
    BASE = (base_output.reshape(B * S, H)
            + np.asarray(o_bias, dtype=np.float32)[None, :])
    cosg = cos[0, 0][pos]                     # [B*S, HD]
    sing = sin[0, 0][pos]
    sr = sing.copy()
    sr[:, :HD // 2] *= -1.0                   # sign for rotate_half product

    inv = 1.0 / np.sqrt(HD)
    gate = float(np.asarray(adaption_gate).reshape(-1)[0])
    q_wT = (np.asarray(q_w, dtype=np.float64)
            * np.asarray(q_scale, dtype=np.float64)[None, :]).T
    k_wT = (np.asarray(k_w, dtype=np.float64)
            * np.asarray(k_scale, dtype=np.float64)[None, :]).T
    v_wT = (np.asarray(v_w, dtype=np.float64)
            * np.asarray(v_scale, dtype=np.float64)[None, :]).T
    o_wT = (np.asarray(o_w, dtype=np.float64)
            * np.asarray(o_scale, dtype=np.float64)[None, :]).T
    pr = np.asarray(adaption_prompt, dtype=np.float64)[0]      # [L, H]

    # weight-only folds (host, f64): adapter K/V and per-head V_h @ Wo_h
    ak = (pr @ k_wT + np.asarray(k_bias, dtype=np.float64)[None, :]) * inv
    av = (pr @ v_wT + np.asarray(v_bias, dtype=np.float64)[None, :]) * gate
    VWO = np.empty((NUM_HEADS * L, H))
    for h in range(NUM_HEADS):
        VWO[h * L:(h + 1) * L, :] = (
            av[:, h * HD:(h + 1) * HD] @ o_wT[h * HD:(h + 1) * HD, :])
    katt_ = np.empty((P, NUM_HEADS * L), dtype=np.float32)
    for h in range(NUM_HEADS):
        katt_[:, h * L:(h + 1) * L] = ak[:, h * HD:(h + 1) * HD].T
    blk = np.zeros((P, P), dtype=np.float32)
    blk[:64, :64] = 1.0
    blk[64:, 64:] = 1.0

    use_fp8 = gate < FP8_GATE_MAX
    variant = "fp8" if use_fp8 else "bf16"
    qb_base = np.asarray(q_bias, dtype=np.float32).reshape(MT, P).T

    if use_fp8:
        sx = 32.0
        sw = _pow2_scale(np.abs(q_wT).max())
        sv = _pow2_scale(np.abs(VWO).max())
        # descale 1/(sx*sw) folded into cos/srot; q bias pre-scaled up
        qb2_ = np.ascontiguousarray(qb_base * (sx * sw)).astype(np.float32)
        qbr_ = np.ascontiguousarray(np.roll(qb2_, -64, axis=0))
        cos_h = (cosg / (sx * sw)).astype(np.float32)
        sr_h = (sr / (sx * sw)).astype(np.float32)
        qw8_ = np.ascontiguousarray(
            (q_wT * sw).reshape(4, 4, 2, P, MT, P).transpose(4, 0, 3, 1, 2, 5)
        ).astype(ml_dtypes.float8_e4m3)
        vw8_ = np.ascontiguousarray(
            (VWO * sv).reshape(4, 2, 2, P, MT, P).transpose(4, 0, 3, 1, 2, 5)
        ).astype(ml_dtypes.float8_e4m3)
        dsc_ = np.full((P, 1), 1.0 / (16.0 * sv), dtype=np.float32)
    else:
        qb2_ = np.ascontiguousarray(qb_base)
        qbr_ = np.ascontiguousarray(np.roll(qb2_, -64, axis=0))
        cos_h = cosg.astype(np.float32)
        sr_h = sr.astype(np.float32)
        qwB_ = np.ascontiguousarray(
            q_wT.reshape(8, 4, P, MT, P).transpose(3, 0, 2, 1, 4)
        ).astype(ml_dtypes.bfloat16)
        vwoB_ = np.ascontiguousarray(
            VWO.reshape(4, 4, P, MT, P).transpose(3, 0, 2, 1, 4)
        ).astype(ml_dtypes.bfloat16)

    katt_b = katt_.astype(ml_dtypes.bfloat16)
    blk_b = blk.astype(ml_dtypes.bfloat16)

    in_maps = []
    for c in range(NC):
        sl = slice(c * TOK, (c + 1) * TOK)
        m = {
            "katt": katt_b,
            "blkones": blk_b,
            "qb2": qb2_,
            "qbr": qbr_,
        }
        if use_fp8:
            xc = np.ascontiguousarray(
                (X[sl].T * sx).reshape(KT // 2, 2, P, TOK).transpose(0, 2, 1, 3)
            ).astype(ml_dtypes.float8_e4m3)
            m.update({
                "xq8": xc,
                "qw8": qw8_,
                "vw8": vw8_,
                "cosT": np.ascontiguousarray(cos_h[sl].T).astype(ml_dtypes.bfloat16),
                "srotT": np.ascontiguousarray(sr_h[sl].T).astype(ml_dtypes.bfloat16),
                "baseT": np.ascontiguousarray(BASE[sl].T).astype(ml_dtypes.bfloat16),
                "dsc": dsc_,
            })
        else:
            m.update({
                "xT": np.ascontiguousarray(X[sl].T).astype(ml_dtypes.bfloat16),
                "qwB": qwB_,
                "vwoB": vwoB_,
                "cosT": np.ascontiguousarray(cos_h[sl].T),
                "srotT": np.ascontiguousarray(sr_h[sl].T),
                "baseT": np.ascontiguousarray(BASE[sl].T),
            })
        in_maps.append(m)

    if variant not in _NC_CACHE:
        _NC_CACHE[variant] = build_fp8() if use_fp8 else build_bf16()
    nc = _NC_CACHE[variant]

    trace = bool(os.environ.get("KERNEL_TRACE"))
    res = run_bass_kernel_spmd(nc, in_maps, core_ids=list(range(NC)),
                               trace=trace)
    if trace and res.exec_time_ns is not None:
        print(f"HW exec time: {res.exec_time_ns} ns")

    out = np.empty((B * S, H), dtype=np.float32)
    for c in range(NC):
        out[c * TOK:(c + 1) * TOK, :] = res.results[c]["outT"].T
    return out.reshape(B, S, H)



# revision 35
# speedup vs baseline: 1.1690x; 1.1690x over previous
"""AdaptedAttention (llama-adapter) Trainium2 kernel, 8-core token-data-parallel.

v4: fp8e4m3 DoubleRow for both big GEMMs, always on (measured rel err
2.5e-3 at the harness gate=0.786, vs the 2e-2 pass bar). Per-MM cost on
this hw is streaming-bound at ~227ns per 512-col pass regardless of
dtype, so DoubleRow's 2 contraction-tiles per pass is a true 2x; the
kernel runs at the MM-count floor (1024 q + 512 p2 + 96 sc/den passes
~= 370us) plus small head/tail.

Structure:
  - 8192 tokens split 1024/core; weights replicated; no collectives.
  - Adapter K/V and per-head V_h @ Wo_h folded on host in f64. Phase-2
    is probs @ VWO with contraction 2048 (32 heads x 64).
  - Head-paired softmax in one PSUM tile; denominators via
    block-diag-ones matmul; probs = exp(sc - ln den + ln 16) into fp8.
  - j-innermost matmul loops + a post-build LDWEIGHTS dedupe pass (each
    weight tile loads once, streams both 512-token halves).
  - Startup: x tiles stream before weights; no gpsimd DMAs (software
    DGE library load otherwise delays kernel start ~3us); first phase-2
    weight tiles prefetch mid-phase-1 from a dedicated pool.
  - Output stored bf16 (halves store traffic), upcast on host.
"""
import os
import math
import numpy as np

import concourse.bass as bass
import concourse.tile as tile
from concourse import mybir
from concourse.bass_utils import run_bass_kernel_spmd

F32 = mybir.dt.float32
BF16 = mybir.dt.bfloat16
F8 = mybir.dt.float8e4
P = 128

NUM_HEADS = 32
B, S, H, L = 4, 2048, 4096, 64
HD = H // NUM_HEADS            # 128
NC = 8
TOK = (B * S) // NC            # 1024 tokens per core
KT = H // P                    # 32 contraction tiles for q-proj
MT = H // P                    # 32 output feature tiles
NPAIR = NUM_HEADS // 2         # 16 head pairs
NB = 2                         # 512-wide matmul chunks (PSUM bank limit)
NW = TOK // NB
# fp8 build measured at rel-err 2.5e-3 on the harness inputs (gate=0.786),
# ~8x inside the 2e-2 gate, and ~1.2x faster than the bf16 build. Use it
# for any gate; env override kept for A/B testing.
FP8_GATE_MAX = float(os.environ.get("KFP8_GATE", "10.0"))

_PATCHED = False


def dedupe_ldweights(nc):
    """Remove InstLdweights that reload the exact weights already resident
    in the PE array (same physical AP + perf_mode, no intervening weight
    change, no sync side effects). The following matmuls then reuse the
    loaded stationary operand; verified correct on hw (microbench)."""
    removed = 0
    skippable = ("InstMatmult", "InstNoOp", "InstEventSemaphore")
    for f in nc.m.functions:
        for b in f.blocks:
            insts = b.instructions
            last_sig = None
            to_remove = []
            for i in insts:
                tn = type(i).__name__
                if tn == "InstLdweights":
                    si = i.sync_info
                    clean = si is None or (not si.on_wait and not si.on_update)
                    sig = (repr(i.ins[0]), str(i.perf_mode), bool(i.is_transpose))
                    if sig == last_sig and clean:
                        to_remove.append(i)
                        continue
                    last_sig = sig
                elif tn in skippable:
                    pass
                elif getattr(i, "engine", None) == mybir.EngineType.PE:
                    last_sig = None
            for i in to_remove:
                insts.remove(i)
            removed += len(to_remove)
    return removed


def _patch_tile():
    """TRN2 instructions have one hw wait slot; walrus rejects multi-wait
    matmuls and the kernel-tail drain. Hoist extra waits onto NoOps."""
    global _PATCHED
    if _PATCHED:
        return
    _PATCHED = True
    import concourse.tile as _tile
    from concourse.vector_clock import ScopedClock

    _orig_commit = _tile.TileContext._commit_instruction

    def _patched_commit(self, inst, lazy_reg_writes=True):
        si = getattr(inst, "sync_info", None)
        if (
            si is not None
            and si.on_wait
            and len(si.on_wait) > 1
            and inst.engine != mybir.EngineType.Unassigned
            and not isinstance(inst, mybir.InstNoOp)
        ):
            waits = list(si.on_wait)
            for w in waits[:-1]:
                nop = mybir.InstNoOp(
                    name=self.nc.get_next_instruction_name(),
                    ins=[], outs=[], bass_nofuse=True,
                )
                nop.engine = inst.engine
                nop.sync_info = mybir.SyncInfo(on_wait=[w], on_update=[])
                _orig_commit(self, nop, lazy_reg_writes=False)
            inst.sync_info = mybir.SyncInfo(
                on_wait=waits[-1:], on_update=list(si.on_update or [])
            )
        return _orig_commit(self, inst, lazy_reg_writes=lazy_reg_writes)

    def _patched_drain_and_barrier(self, tick_clock, wait_clock):
        nc = self.nc
        carrier = nc.sync.nop(nofuse=True)
        wait_clock.add_sem_waits(
            carrier.ins, ScopedClock({None: tick_clock.global_clock})
        )
        si = carrier.ins.sync_info
        waits = list(si.on_wait) if si and si.on_wait else []
        if len(waits) > 1:
            carrier.ins.sync_info = mybir.SyncInfo(
                on_wait=waits[:1], on_update=list(si.on_update or [])
            )
            for w in waits[1:]:
                extra = nc.sync.nop(nofuse=True)
                extra.ins.sync_info = mybir.SyncInfo(on_wait=[w], on_update=[])
        nc.sync.drain()
        nc.all_engine_barrier()
        assert self.sems is not None
        popped = nc._tile_sem_poison_stack.pop()
        assert popped is self._sem_poison
        nc.clear_and_free_semaphores(list(self.sems.allocated().values()))
        nc.all_engine_barrier()

    _tile.TileContext._commit_instruction = _patched_commit
    _tile.TileContext._drain_and_barrier = _patched_drain_and_barrier


def build_fp8():
    _patch_tile()
    from contextlib import ExitStack

    nc = bass.Bass(target_bir_lowering=False)

    xq8 = nc.declare_dram_parameter("xq8", [KT // 2, P, 2, TOK], F8, isOutput=False)
    qw8 = nc.declare_dram_parameter("qw8", [MT, 4, P, 4, 2, P], F8, isOutput=False)
    vw8 = nc.declare_dram_parameter("vw8", [MT, 4, P, 2, 2, P], F8, isOutput=False)
    katt = nc.declare_dram_parameter("katt", [P, NUM_HEADS * L], BF16, isOutput=False)
    blkones = nc.declare_dram_parameter("blkones", [P, P], BF16, isOutput=False)
    cosT = nc.declare_dram_parameter("cosT", [HD, TOK], BF16, isOutput=False)
    srotT = nc.declare_dram_parameter("srotT", [HD, TOK], BF16, isOutput=False)
    qb2 = nc.declare_dram_parameter("qb2", [P, MT], F32, isOutput=False)
    qbr = nc.declare_dram_parameter("qbr", [P, MT], F32, isOutput=False)
    baseT = nc.declare_dram_parameter("baseT", [H, TOK], BF16, isOutput=False)
    dsc = nc.declare_dram_parameter("dsc", [P, 1], F32, isOutput=False)
    outT = nc.declare_dram_parameter("outT", [H, TOK], BF16, isOutput=True)

    LN16 = float(np.log(16.0))

    with tile.TileContext(nc) as tc:
        es = ExitStack()
        persist = es.enter_context(tc.tile_pool(name="persist", bufs=1))

        ph_es = ExitStack()
        psQ = ph_es.enter_context(tc.tile_pool(name="psQ", bufs=4, space="PSUM"))
        psS = ph_es.enter_context(tc.tile_pool(name="psS", bufs=2, space="PSUM"))
        wpool = ph_es.enter_context(tc.tile_pool(name="wpool", bufs=16))
        probs_pool = ph_es.enter_context(tc.tile_pool(name="probs", bufs=1))
        # dedicated pool for the first phase-2 weight tiles (prefetched
        # mid-phase-1); must be created before the xt-scope pools so pool
        # releases stay LIFO
        vpool = ph_es.enter_context(tc.tile_pool(name="vpool", bufs=12))
        xt_es = ExitStack()
        xt_pool = xt_es.enter_context(tc.tile_pool(name="xt", bufs=1))
        rpool = xt_es.enter_context(tc.tile_pool(name="rpool", bufs=2))
        tpool = xt_es.enter_context(tc.tile_pool(name="tpool", bufs=2))

        # ---- urgent streams, interleaved by first-use: the first matmuls
        # need xq d0-d3 + the first head's 4 weight tiles, so those lead;
        # the rest follow. gpsimd DMAs avoided everywhere: software DGE
        # pulls in a library load that delays kernel start by ~3us.
        xq_tiles = [None] * (KT // 2)
        qw_pref = {}

        def _load_xq(d, eng):
            t = xt_pool.tile([P, 2, TOK], F8, name=f"xq_{d}", tag=f"xq_{d}")
            eng.dma_start(out=t[:], in_=xq8[d])
            xq_tiles[d] = t

        def _load_qw(h, c, eng):
            wb = wpool.tile([P, 4, 2, P], F8, name=f"qw_{h}_{c}", tag="wq")
            eng.dma_start(out=wb[:], in_=qw8[h, c])
            qw_pref[(h, c)] = wb

        # heads are processed in rotated order HSEQ (see below): first two
        # processed heads are 4 and 5
        _load_xq(0, nc.sync)
        _load_xq(1, nc.scalar)
        _load_qw(4, 0, nc.sync)
        _load_qw(4, 1, nc.scalar)
        _load_xq(2, nc.sync)
        _load_xq(3, nc.scalar)
        _load_qw(4, 2, nc.sync)
        _load_qw(4, 3, nc.scalar)
        for d in range(4, KT // 2):
            _load_xq(d, (nc.sync, nc.scalar)[d % 2])
        for c in range(4):
            _load_qw(5, c, (nc.sync, nc.scalar)[c % 2])

        cos_sb = persist.tile([HD, TOK], BF16, name="cos_sb")
        nc.sync.dma_start(out=cos_sb[:], in_=cosT[:])
        srot_sb = persist.tile([HD, TOK], BF16, name="srot_sb")
        nc.scalar.dma_start(out=srot_sb[:], in_=srotT[:])
        katt_sb = persist.tile([P, NUM_HEADS * L], BF16, name="katt_sb")
        nc.sync.dma_start(out=katt_sb[:], in_=katt[:])
        blk_sb = persist.tile([P, P], BF16, name="blk_sb")
        nc.sync.dma_start(out=blk_sb[:], in_=blkones[:])
        qb2_sb = persist.tile([P, MT], F32, name="qb2_sb")
        nc.scalar.dma_start(out=qb2_sb[:], in_=qb2[:])
        qbr_sb = persist.tile([P, MT], F32, name="qbr_sb")
        nc.scalar.dma_start(out=qbr_sb[:], in_=qbr[:])
        dsc_sb = persist.tile([P, 1], F32, name="dsc_sb")
        nc.sync.dma_start(out=dsc_sb[:], in_=dsc[:])
        ln16_sb = persist.tile([P, 1], F32, name="ln16_sb")
        nc.vector.memset(ln16_sb[:], LN16)
        s16_sb = persist.tile([P, 1], F32, name="s16_sb")
        nc.vector.memset(s16_sb[:], 16.0)

        probs_quads = [
            probs_pool.tile([P, 2, TOK], F8, name=f"probs_{q}", tag=f"probs_{q}")
            for q in range(NPAIR // 2)
        ]

        sc_pair = [None]
        expt_tiles = {}
        den_tiles = {}

        def emit_q(h, pqs=None):
            # per-j [P, NW] psum tiles on a 4-deep ring: each half frees as
            # soon as its own drain copy runs, so the next q block never
            # waits on the slower half's reader
            if pqs is None:
                pqs = [psQ.tile([P, NW], F32, name=f"pq_{h}_{j}", tag="psQ",
                                bufs=4) for j in range(NB)]
            wbs = []
            for c in range(4):
                if (h, c) in qw_pref:
                    wb = qw_pref[(h, c)]
                else:
                    wb = wpool.tile([P, 4, 2, P], F8, name=f"qw_{h}_{c}", tag="wq")
                    eng = (nc.sync, nc.scalar)[(h * 4 + c) % 2]
                    eng.dma_start(out=wb[:], in_=qw8[h, c])
                wbs.append(wb)
            # j innermost: both 512-token halves stream against one weight
            # load (the dedupe pass drops the second LDWEIGHTS)
            for c in range(4):
                for dd in range(4):
                    d = c * 4 + dd
                    for j in range(NB):
                        nc.tensor.matmul(
                            out=pqs[j][:],
                            lhsT=wbs[c][:, dd, :, :],
                            rhs=xq_tiles[d][:, :, j * NW:(j + 1) * NW],
                            start=(d == 0), stop=(d == KT // 2 - 1),
                            perf_mode=mybir.MatmulPerfMode.DoubleRow,
                        )
            return pqs

        def emit_rope(h, pqs):
            q_sb = rpool.tile([P, TOK], BF16, name=f"q_sb_{h}", tag="q_sb")
            # split the PSUM drain across two engines so each half's buffer
            # frees independently
            nc.scalar.copy(out=q_sb[:, :NW], in_=pqs[0][:])
            nc.vector.tensor_copy(out=q_sb[:, NW:], in_=pqs[1][:])
            rot = rpool.tile([P, TOK], BF16, name=f"rot_{h}", tag="rot")
            nc.scalar.dma_start(out=rot[0:64, :], in_=q_sb[64:128, :])
            nc.sync.dma_start(out=rot[64:128, :], in_=q_sb[0:64, :])
            t2 = rpool.tile([P, TOK], BF16, name=f"t2_{h}", tag="t2")
            nc.vector.scalar_tensor_tensor(
                out=t2[:], in0=q_sb[:], scalar=qb2_sb[:, h:h + 1],
                in1=cos_sb[:], op0=mybir.AluOpType.add,
                op1=mybir.AluOpType.mult)
            t1 = rpool.tile([P, TOK], BF16, name=f"t1_{h}", tag="t1")
            nc.vector.scalar_tensor_tensor(
                out=t1[:], in0=rot[:], scalar=qbr_sb[:, h:h + 1],
                in1=srot_sb[:], op0=mybir.AluOpType.add,
                op1=mybir.AluOpType.mult)
            qr = rpool.tile([P, TOK], BF16, name=f"qr_{h}", tag="qr", bufs=3)
            nc.vector.tensor_add(out=qr[:], in0=t2[:], in1=t1[:])
            return qr

        qr_even = [None]

        def emit_sc(h, qr):
            # defer the even head; emit both heads' score matmuls together
            # so the two 64-partition outputs run CONCURRENTLY in separate
            # PE column groups (tile_position derives from out partition
            # offset) — 4 MMs in ~2 MM times instead of 4
            hh = h % 2
            p = h // 2
            if hh == 0:
                qr_even[0] = qr
                return
            sc = psS.tile([P, TOK], F32, name=f"sc_{p}", tag="psS")
            sc_pair[0] = sc
            for j in range(NB):
                for hh2, qrx in ((0, qr_even[0]), (1, qr)):
                    hx = 2 * p + hh2
                    nc.tensor.matmul(
                        out=sc[hh2 * 64:(hh2 + 1) * 64, j * NW:(j + 1) * NW],
                        lhsT=katt_sb[:, hx * L:(hx + 1) * L],
                        rhs=qrx[:, j * NW:(j + 1) * NW],
                        start=True, stop=True,
                    )
            expt = tpool.tile([P, TOK], BF16, name=f"expt_{p}",
                              tag="expt", bufs=2)
            nc.scalar.activation(expt[:], sc[:],
                                 mybir.ActivationFunctionType.Exp)
            expt_tiles[p] = (expt, sc)

        def emit_den(p, fast_tail=False):
            expt, scp = expt_tiles.pop(p)
            dens = [psQ.tile([P, NW], F32, name=f"den_{p}_{j}", tag="psQ",
                             bufs=4) for j in range(NB)]
            for j in range(NB):
                nc.tensor.matmul(
                    out=dens[j][:],
                    lhsT=blk_sb[:],
                    rhs=expt[:, j * NW:(j + 1) * NW],
                    start=True, stop=True,
                )
            # NOTE: a vector.reciprocal softmax tail was tried here and is
            # ~3.3us per 512-col half on DVE — far slower than this chain.
            lnden = tpool.tile([P, TOK], F32, name=f"lnden_{p}",
                               tag="lnden", bufs=2)
            d2 = tpool.tile([P, TOK], F32, name=f"d2_{p}", tag="d2", bufs=2)
            if fast_tail:
                # last pair: fully j-split ln->sub->exp chain so the j0
                # half of the quad is ready ~1.2us earlier for phase 2
                for j in range(NB):
                    sl = slice(j * NW, (j + 1) * NW)
                    nc.scalar.activation(lnden[:, sl], dens[j][:],
                                         mybir.ActivationFunctionType.Ln)
                    nc.vector.tensor_sub(out=d2[:, sl], in0=scp[:, sl],
                                         in1=lnden[:, sl])
                    nc.scalar.activation(
                        probs_quads[p // 2][:, p % 2, sl], d2[:, sl],
                        mybir.ActivationFunctionType.Exp,
                        bias=ln16_sb[:, 0:1])
                return
            for j in range(NB):
                nc.scalar.activation(lnden[:, j * NW:(j + 1) * NW],
                                     dens[j][:],
                                     mybir.ActivationFunctionType.Ln)
            nc.vector.tensor_sub(out=d2[:], in0=scp[:], in1=lnden[:])
            nc.scalar.activation(
                probs_quads[p // 2][:, p % 2, :], d2[:],
                mybir.ActivationFunctionType.Exp, bias=ln16_sb[:, 0:1])

        # Heads processed in rotated order: heads 0..3 (probs quad 0) come
        # LAST, and phase 2 contracts quad 0 last, so the final heads'
        # softmax chain (a ~5us scalar/vector serial tail) hides behind
        # phase-2 matmuls on quads 1..7.
        HSEQ = list(range(4, NUM_HEADS)) + [0, 1, 2, 3]
        PSEQ = [h // 2 for h in HSEQ[::2]]  # pair completion order

        # first phase-2 weight tiles prefetch mid-phase-1 from vpool so
        # their DMAs are not gated on the q-weight ring freeing (that
        # wait was a 4.4us PE stall at the phase boundary)
        vwo_pref = {}
        m0_pos = [None]
        m0_wbs = [None]

        # NOTE: a HAM warm-up (48 zero matmuls during the initial DMA wait)
        # was tried here and regressed exec by ~70us — the repeated
        # single-MM start/stop groups serialize pathologically. Don't.
        pq_of = {}
        for step in range(NUM_HEADS):
            pq_of[HSEQ[step]] = emit_q(HSEQ[step])
            if step == 26:
                for m in range(3):
                    for c in range(4):
                        wb = vpool.tile([P, 2, 2, P], F8,
                                        name=f"vw_{m}_{c}", tag="wv0")
                        eng = (nc.sync, nc.scalar)[(m * 4 + c) % 2]
                        eng.dma_start(out=wb[:], in_=vw8[m, c])
                        vwo_pref[(m, c)] = wb
            if step >= 1:
                h = HSEQ[step - 1]
                qr = emit_rope(h, pq_of.pop(h))
                emit_sc(h, qr)
            if step >= 4 and (step % 2) == 0:
                emit_den(PSEQ[(step - 4) // 2])

        # ---- explicit phase-1 tail, ordered to keep the PE fed while the
        # last pairs' softmax drains on scalar/vector:
        #   rope(3) queued first (vector chain overlaps the PE work below)
        #   den(p0), then m0's quads 1..7 (3.2us of PE work),
        #   sc pair p1, then den(p1) via the vector-only recip path.
        h_last = HSEQ[-1]
        qr_last = emit_rope(h_last, pq_of.pop(h_last))
        emit_den(PSEQ[14])
        pos = [psQ.tile([P, NW], F32, name=f"po_0_{j}", tag="psQ",
                        bufs=4) for j in range(NB)]
        wbs = [vwo_pref[(0, c)] for c in range(4)]
        for q in range(1, NPAIR // 2):
            for j in range(NB):
                nc.tensor.matmul(
                    out=pos[j][:],
                    lhsT=wbs[q // 2][:, q % 2, :, :],
                    rhs=probs_quads[q][:, :, j * NW:(j + 1) * NW],
                    start=(q == 1), stop=False,
                    perf_mode=mybir.MatmulPerfMode.DoubleRow,
                )
        m0_pos[0] = pos
        m0_wbs[0] = wbs
        emit_sc(h_last, qr_last)
        emit_den(PSEQ[15])

        xt_es.close()

        # ---- phase 2: out = probs @ VWO (DR) * dsc + base ----
        p2_es = ExitStack()
        bpool = p2_es.enter_context(tc.tile_pool(name="bpool", bufs=4))

        # quad 0 contracts last (its probs complete last under HSEQ)
        QSEQ = list(range(1, NPAIR // 2)) + [0]
        for m in range(MT):
            if m == 0:
                # quads 1..7 already emitted at the phase boundary
                pos, wbs = m0_pos[0], m0_wbs[0]
                qlist = [0]
            else:
                pos = [psQ.tile([P, NW], F32, name=f"po_{m}_{j}", tag="psQ",
                                bufs=4) for j in range(NB)]
                wbs = []
                for c in range(4):
                    if (m, c) in vwo_pref:
                        wb = vwo_pref[(m, c)]
                    else:
                        wb = wpool.tile([P, 2, 2, P], F8, name=f"vw_{m}_{c}",
                                        tag="wv")
                        eng = (nc.sync, nc.scalar)[(m * 4 + c) % 2]
                        eng.dma_start(out=wb[:], in_=vw8[m, c])
                    wbs.append(wb)
                qlist = QSEQ
            bt = bpool.tile([P, TOK], BF16, name=f"bt_{m}", tag="bt")
            nc.scalar.dma_start(out=bt[:], in_=baseT[m * P:(m + 1) * P, :])
            for q in qlist:
                for j in range(NB):
                    nc.tensor.matmul(
                        out=pos[j][:],
                        lhsT=wbs[q // 2][:, q % 2, :, :],
                        rhs=probs_quads[q][:, :, j * NW:(j + 1) * NW],
                        start=(m != 0 and q == QSEQ[0]), stop=(q == 0),
                        perf_mode=mybir.MatmulPerfMode.DoubleRow,
                    )
            os_ = bpool.tile([P, TOK], BF16, name=f"os_{m}", tag="os")
            for j in range(NB):
                nc.vector.scalar_tensor_tensor(
                    out=os_[:, j * NW:(j + 1) * NW], in0=pos[j][:],
                    scalar=dsc_sb[:, 0:1],
                    in1=bt[:, j * NW:(j + 1) * NW], op0=mybir.AluOpType.mult,
                    op1=mybir.AluOpType.add)
                (nc.sync, nc.scalar)[(2 * m + j) % 2].dma_start(
                    out=outT[m * P:(m + 1) * P, j * NW:(j + 1) * NW],
                    in_=os_[:, j * NW:(j + 1) * NW])

        p2_es.close()
        ph_es.close()
        es.close()
    dedupe_ldweights(nc)
    return nc


def build_bf16():
    _patch_tile()
    from contextlib import ExitStack

    nc = bass.Bass(target_bir_lowering=False)

    xT = nc.declare_dram_parameter("xT", [H, TOK], BF16, isOutput=False)
    qwB = nc.declare_dram_parameter("qwB", [MT, 8, P, 4, P], BF16, isOutput=False)
    vwoB = nc.declare_dram_parameter("vwoB", [MT, 4, P, 4, P], BF16, isOutput=False)
    katt = nc.declare_dram_parameter("katt", [P, NUM_HEADS * L], BF16, isOutput=False)
    blkones = nc.declare_dram_parameter("blkones", [P, P], BF16, isOutput=False)
    cosT = nc.declare_dram_parameter("cosT", [HD, TOK], F32, isOutput=False)
    srotT = nc.declare_dram_parameter("srotT", [HD, TOK], F32, isOutput=False)
    qb2 = nc.declare_dram_parameter("qb2", [P, MT], F32, isOutput=False)
    qbr = nc.declare_dram_parameter("qbr", [P, MT], F32, isOutput=False)
    baseT = nc.declare_dram_parameter("baseT", [H, TOK], F32, isOutput=False)
    outT = nc.declare_dram_parameter("outT", [H, TOK], F32, isOutput=True)

    with tile.TileContext(nc) as tc:
        es = ExitStack()
        persist = es.enter_context(tc.tile_pool(name="persist", bufs=1))

        ph_es = ExitStack()
        psQ = ph_es.enter_context(tc.tile_pool(name="psQ", bufs=2, space="PSUM"))
        psS = ph_es.enter_context(tc.tile_pool(name="psS", bufs=1, space="PSUM"))
        psD = ph_es.enter_context(tc.tile_pool(name="psD", bufs=1, space="PSUM"))
        wpool = ph_es.enter_context(tc.tile_pool(name="wpool", bufs=20))
        probs_pool = ph_es.enter_context(tc.tile_pool(name="probs", bufs=1))
        xt_es = ExitStack()
        xt_pool = xt_es.enter_context(tc.tile_pool(name="xt", bufs=1))
        rpool = xt_es.enter_context(tc.tile_pool(name="rpool", bufs=2))
        tpool = xt_es.enter_context(tc.tile_pool(name="tpool", bufs=2))

        qw_pref = {}
        for h in range(2):
            for kg in range(8):
                wb = wpool.tile([P, 4, P], BF16, name=f"qw_{h}_{kg}", tag="wq")
                eng = (nc.sync, nc.scalar, nc.gpsimd)[kg % 3]
                eng.dma_start(out=wb[:], in_=qwB[h, kg])
                qw_pref[(h, kg)] = wb
        xt_tiles = []
        for k in range(KT):
            t = xt_pool.tile([P, TOK], BF16, name=f"xt_{k}", tag=f"xt_{k}")
            eng = (nc.sync, nc.scalar, nc.gpsimd)[k % 3]
            eng.dma_start(out=t[:], in_=xT[k * P:(k + 1) * P, :])
            xt_tiles.append(t)

        cos_sb = persist.tile([HD, TOK], F32, name="cos_sb")
        nc.gpsimd.dma_start(out=cos_sb[:], in_=cosT[:])
        srot_sb = persist.tile([HD, TOK], F32, name="srot_sb")
        nc.scalar.dma_start(out=srot_sb[:], in_=srotT[:])
        katt_sb = persist.tile([P, NUM_HEADS * L], BF16, name="katt_sb")
        nc.sync.dma_start(out=katt_sb[:], in_=katt[:])
        blk_sb = persist.tile([P, P], BF16, name="blk_sb")
        nc.sync.dma_start(out=blk_sb[:], in_=blkones[:])
        qb2_sb = persist.tile([P, MT], F32, name="qb2_sb")
        nc.gpsimd.dma_start(out=qb2_sb[:], in_=qb2[:])
        qbr_sb = persist.tile([P, MT], F32, name="qbr_sb")
        nc.scalar.dma_start(out=qbr_sb[:], in_=qbr[:])

        probs_tiles = [
            probs_pool.tile([P, TOK], BF16, name=f"probs_{p}", tag=f"probs_{p}")
            for p in range(NPAIR)
        ]

        sc_pair = [None]
        expt_tiles = {}

        def emit_q(h):
            pq = psQ.tile([P, TOK], F32, name=f"pq_{h}", tag="psQ")
            wbs = []
            for kg in range(8):
                if (h, kg) in qw_pref:
                    wb = qw_pref[(h, kg)]
                else:
                    wb = wpool.tile([P, 4, P], BF16, name=f"qw_{h}_{kg}", tag="wq")
                    eng = (nc.sync, nc.scalar, nc.gpsimd)[(h * 8 + kg) % 3]
                    eng.dma_start(out=wb[:], in_=qwB[h, kg])
                wbs.append(wb)
            for j in range(NB):
                for kg in range(8):
                    for kk in range(4):
                        k = kg * 4 + kk
                        nc.tensor.matmul(
                            out=pq[:, j * NW:(j + 1) * NW],
                            lhsT=wbs[kg][:, kk, :],
                            rhs=xt_tiles[k][:, j * NW:(j + 1) * NW],
                            start=(k == 0), stop=(k == KT - 1),
                        )
            return pq

        def emit_rope(h, pq):
            q_sb = rpool.tile([P, TOK], BF16, name=f"q_sb_{h}", tag="q_sb")
            nc.scalar.copy(out=q_sb[:], in_=pq[:])
            rot = rpool.tile([P, TOK], BF16, name=f"rot_{h}", tag="rot")
            nc.scalar.dma_start(out=rot[0:64, :], in_=q_sb[64:128, :])
            nc.sync.dma_start(out=rot[64:128, :], in_=q_sb[0:64, :])
            t2 = rpool.tile([P, TOK], F32, name=f"t2_{h}", tag="t2")
            nc.vector.scalar_tensor_tensor(
                out=t2[:], in0=q_sb[:], scalar=qb2_sb[:, h:h + 1],
                in1=cos_sb[:], op0=mybir.AluOpType.add,
                op1=mybir.AluOpType.mult)
            t1 = rpool.tile([P, TOK], F32, name=f"t1_{h}", tag="t1")
            nc.vector.scalar_tensor_tensor(
                out=t1[:], in0=rot[:], scalar=qbr_sb[:, h:h + 1],
                in1=srot_sb[:], op0=mybir.AluOpType.add,
                op1=mybir.AluOpType.mult)
            qr = rpool.tile([P, TOK], BF16, name=f"qr_{h}", tag="qr", bufs=3)
            nc.vector.tensor_add(out=qr[:], in0=t2[:], in1=t1[:])
            return qr

        def emit_sc(h, qr):
            hh = h % 2
            p = h // 2
            if hh == 0:
                sc_pair[0] = psS.tile([P, TOK], F32, name=f"sc_{p}", tag="psS")
            for j in range(NB):
                nc.tensor.matmul(
                    out=sc_pair[0][hh * 64:(hh + 1) * 64, j * NW:(j + 1) * NW],
                    lhsT=katt_sb[:, h * L:(h + 1) * L],
                    rhs=qr[:, j * NW:(j + 1) * NW],
                    start=True, stop=True,
                )
            if hh == 1:
                expt = tpool.tile([P, TOK], BF16, name=f"expt_{p}",
                                  tag="expt", bufs=2)
                nc.scalar.activation(expt[:], sc_pair[0][:],
                                     mybir.ActivationFunctionType.Exp)
                expt_tiles[p] = expt

        def emit_den(p):
            expt = expt_tiles.pop(p)
            den = psD.tile([P, TOK], F32, name=f"den_{p}", tag="psD")
            for j in range(NB):
                nc.tensor.matmul(
                    out=den[:, j * NW:(j + 1) * NW],
                    lhsT=blk_sb[:],
                    rhs=expt[:, j * NW:(j + 1) * NW],
                    start=True, stop=True,
                )
            recip = tpool.tile([P, TOK], F32, name=f"recip_{p}",
                               tag="recip", bufs=2)
            nc.vector.reciprocal(out=recip[:], in_=den[:])
            nc.vector.tensor_mul(out=probs_tiles[p][:], in0=expt[:],
                                 in1=recip[:])

        pq_of = {}
        for step in range(NUM_HEADS + 2):
            if step < NUM_HEADS:
                pq_of[step] = emit_q(step)
            if step >= 3 and (step % 2) == 1:
                emit_den((step - 3) // 2)
            if 1 <= step <= NUM_HEADS:
                h = step - 1
                qr = emit_rope(h, pq_of.pop(h))
                emit_sc(h, qr)

        vwo_pref = {}
        for m in range(2):
            for g in range(4):
                wb = wpool.tile([P, 4, P], BF16, name=f"vw_{m}_{g}", tag="wq")
                eng = (nc.sync, nc.scalar, nc.gpsimd)[(m * 4 + g) % 3]
                eng.dma_start(out=wb[:], in_=vwoB[m, g])
                vwo_pref[(m, g)] = wb

        xt_es.close()

        p2_es = ExitStack()
        bpool = p2_es.enter_context(tc.tile_pool(name="bpool", bufs=3))

        for m in range(MT):
            po = psQ.tile([P, TOK], F32, name=f"po_{m}", tag="psQ")
            wbs = []
            for g in range(4):
                if (m, g) in vwo_pref:
                    wb = vwo_pref[(m, g)]
                else:
                    wb = wpool.tile([P, 4, P], BF16, name=f"vw_{m}_{g}", tag="wq")
                    eng = (nc.sync, nc.scalar, nc.gpsimd)[(m * 4 + g) % 3]
                    eng.dma_start(out=wb[:], in_=vwoB[m, g])
                wbs.append(wb)
            for j in range(NB):
                for g in range(4):
                    for i in range(4):
                        pr = g * 4 + i
                        nc.tensor.matmul(
                            out=po[:, j * NW:(j + 1) * NW],
                            lhsT=wbs[g][:, i, :],
                            rhs=probs_tiles[pr][:, j * NW:(j + 1) * NW],
                            start=(pr == 0), stop=(pr == NPAIR - 1),
                        )
            bt = bpool.tile([P, TOK], F32, name=f"bt_{m}", tag="bt")
            nc.scalar.dma_start(out=bt[:], in_=baseT[m * P:(m + 1) * P, :])
            os_ = bpool.tile([P, TOK], F32, name=f"os_{m}", tag="os")
            nc.vector.tensor_add(out=os_[:], in0=po[:], in1=bt[:])
            (nc.sync, nc.scalar)[m % 2].dma_start(
                out=outT[m * P:(m + 1) * P, :], in_=os_[:])

        p2_es.close()
        ph_es.close()
        es.close()
    return nc


_NC_CACHE = {}


def _pow2_scale(absmax, target=224.0):
    return 2.0 ** math.floor(math.log2(target / max(absmax, 1e-30)))


def kernel(hidden_states, position_ids, base_output, cos, sin,
           q_w, k_w, v_w, o_w,
           q_scale, k_scale, v_scale, o_scale,
           q_bias, k_bias, v_bias, o_bias,
           adaption_prompt, adaption_gate):
    import ml_dtypes

    hidden_states = np.asarray(hidden_states, dtype=np.float32)
    base_output = np.asarray(base_output, dtype=np.float32)
    pos = np.asarray(position_ids).reshape(-1).astype(np.int64)
    cos = np.asarray(cos, dtype=np.float32)
    sin = np.asarray(sin, dtype=np.float32)

    X = hidden_states.reshape(B * S, H)
    BASE = (base_output.reshape(B * S, H)
            + np.asarray(o_bias, dtype=np.float32)[None, :])
    cosg = cos[0, 0][pos]                     # [B*S, HD]
    sing = sin[0, 0][pos]
    sr = sing.copy()
    sr[:, :HD // 2] *= -1.0                   # sign for rotate_half product

    inv = 1.0 / np.sqrt(HD)
    gate = float(np.asarray(adaption_gate).reshape(-1)[0])
    q_wT = (np.asarray(q_w, dtype=np.float64)
            * np.asarray(q_scale, dtype=np.float64)[None, :]).T
    k_wT = (np.asarray(k_w, dtype=np.float64)
            * np.asarray(k_scale, dtype=np.float64)[None, :]).T
    v_wT = (np.asarray(v_w, dtype=np.float64)
            * np.asarray(v_scale, dtype=np.float64)[None, :]).T
    o_wT = (np.asarray(o_w, dtype=np.float64)
            * np.asarray(o_scale, dtype=np.float64)[None, :]).T
    pr = np.asarray(adaption_prompt, dtype=np.float64)[0]      # [L, H]

    # weight-only folds (host, f64): adapter K/V and per-head V_h @ Wo_h
    ak = (pr @ k_wT + np.asarray(k_bias, dtype=np.float64)[None, :]) * inv
    av = (pr @ v_wT + np.asarray(v_bias, dtype=np.float64)[None, :]) * gate
    VWO = np.empty((NUM_HEADS * L, H))
    for h in range(NUM_HEADS):
        VWO[h * L:(h + 1) * L, :] = (
            av[:, h * HD:(h + 1) * HD] @ o_wT[h * HD:(h + 1) * HD, :])
    katt_ = np.empty((P, NUM_HEADS * L), dtype=np.float32)
    for h in range(NUM_HEADS):
        katt_[:, h * L:(h + 1) * L] = ak[:, h * HD:(h + 1) * HD].T
    blk = np.zeros((P, P), dtype=np.float32)
    blk[:64, :64] = 1.0
    blk[64:, 64:] = 1.0

    use_fp8 = gate < FP8_GATE_MAX
    variant = "fp8" if use_fp8 else "bf16"
    qb_base = np.asarray(q_bias, dtype=np.float32).reshape(MT, P).T

    if use_fp8:
        sx = 32.0
        sw = _pow2_scale(np.abs(q_wT).max())
        sv = _pow2_scale(np.abs(VWO).max())
        # descale 1/(sx*sw) folded into cos/srot; q bias pre-scaled up
        qb2_ = np.ascontiguousarray(qb_base * (sx * sw)).astype(np.float32)
        qbr_ = np.ascontiguousarray(np.roll(qb2_, -64, axis=0))
        cos_h = (cosg / (sx * sw)).astype(np.float32)
        sr_h = (sr / (sx * sw)).astype(np.float32)
        qw8_ = np.ascontiguousarray(
            (q_wT * sw).reshape(4, 4, 2, P, MT, P).transpose(4, 0, 3, 1, 2, 5)
        ).astype(ml_dtypes.float8_e4m3)
        vw8_ = np.ascontiguousarray(
            (VWO * sv).reshape(4, 2, 2, P, MT, P).transpose(4, 0, 3, 1, 2, 5)
        ).astype(ml_dtypes.float8_e4m3)
        dsc_ = np.full((P, 1), 1.0 / (16.0 * sv), dtype=np.float32)
    else:
        qb2_ = np.ascontiguousarray(qb_base)
        qbr_ = np.ascontiguousarray(np.roll(qb2_, -64, axis=0))
        cos_h = cosg.astype(np.float32)
        sr_h = sr.astype(np.float32)
        qwB_ = np.ascontiguousarray(
            q_wT.reshape(8, 4, P, MT, P).transpose(3, 0, 2, 1, 4)
        ).astype(ml_dtypes.bfloat16)
        vwoB_ = np.ascontiguousarray(
            VWO.reshape(4, 4, P, MT, P).transpose(3, 0, 2, 1, 4)
        ).astype(ml_dtypes.bfloat16)

    katt_b = katt_.astype(ml_dtypes.bfloat16)
    blk_b = blk.astype(ml_dtypes.bfloat16)

    in_maps = []
    for c in range(NC):
        sl = slice(c * TOK, (c + 1) * TOK)
        m = {
            "katt": katt_b,
            "blkones": blk_b,
            "qb2": qb2_,
            "qbr": qbr_,
        }
        if use_fp8:
            xc = np.ascontiguousarray(
                (X[sl].T * sx).reshape(KT // 2, 2, P, TOK).transpose(0, 2, 1, 3)
            ).astype(ml_dtypes.float8_e4m3)
            m.update({
                "xq8": xc,
                "qw8": qw8_,
                "vw8": vw8_,
                "cosT": np.ascontiguousarray(cos_h[sl].T).astype(ml_dtypes.bfloat16),
                "srotT": np.ascontiguousarray(sr_h[sl].T).astype(ml_dtypes.bfloat16),
                "baseT": np.ascontiguousarray(BASE[sl].T).astype(ml_dtypes.bfloat16),
                "dsc": dsc_,
            })
        else:
            m.update({
                "xT": np.ascontiguousarray(X[sl].T).astype(ml_dtypes.bfloat16),
                "qwB": qwB_,
                "vwoB": vwoB_,
                "cosT": np.ascontiguousarray(cos_h[sl].T),
                "srotT": np.ascontiguousarray(sr_h[sl].T),
                "baseT": np.ascontiguousarray(BASE[sl].T),
            })
        in_maps.append(m)

    if variant not in _NC_CACHE:
        _NC_CACHE[variant] = build_fp8() if use_fp8 else build_bf16()
    nc = _NC_CACHE[variant]

    trace = bool(os.environ.get("KERNEL_TRACE"))
    res = run_bass_kernel_spmd(nc, in_maps, core_ids=list(range(NC)),
                               trace=trace)
    if trace and res.exec_time_ns is not None:
        print(f"HW exec time: {res.exec_time_ns} ns")

    out = np.empty((B * S, H), dtype=np.float32)
    for c in range(NC):
        out[c * TOK:(c + 1) * TOK, :] = res.results[c]["outT"].T
    return out.reshape(B, S, H)



# revision 37
# speedup vs baseline: 1.1905x; 1.0184x over previous
"""AdaptedAttention (llama-adapter) Trainium2 kernel, 8-core token-data-parallel.

v4: fp8e4m3 DoubleRow for both big GEMMs, always on (measured rel err
2.5e-3 at the harness gate=0.786, vs the 2e-2 pass bar). Per-MM cost on
this hw is streaming-bound at ~227ns per 512-col pass regardless of
dtype, so DoubleRow's 2 contraction-tiles per pass is a true 2x; the
kernel runs at the MM-count floor (1024 q + 512 p2 + 96 sc/den passes
~= 370us) plus small head/tail.

Structure:
  - 8192 tokens split 1024/core; weights replicated; no collectives.
  - Adapter K/V and per-head V_h @ Wo_h folded on host in f64. Phase-2
    is probs @ VWO with contraction 2048 (32 heads x 64).
  - Head-paired softmax in one PSUM tile; denominators via
    block-diag-ones matmul; probs = exp(sc - ln den + ln 16) into fp8.
  - j-innermost matmul loops + a post-build LDWEIGHTS dedupe pass (each
    weight tile loads once, streams both 512-token halves).
  - Startup: x tiles stream before weights; no gpsimd DMAs (software
    DGE library load otherwise delays kernel start ~3us); first phase-2
    weight tiles prefetch mid-phase-1 from a dedicated pool.
  - Output stored bf16 (halves store traffic), upcast on host.
"""
import os
import math
import numpy as np

import concourse.bass as bass
import concourse.tile as tile
from concourse import mybir
from concourse.bass_utils import run_bass_kernel_spmd

F32 = mybir.dt.float32
BF16 = mybir.dt.bfloat16
F8 = mybir.dt.float8e4
P = 128

NUM_HEADS = 32
B, S, H, L = 4, 2048, 4096, 64
HD = H // NUM_HEADS            # 128
NC = 8
TOK = (B * S) // NC            # 1024 tokens per core
KT = H // P                    # 32 contraction tiles for q-proj
MT = H // P                    # 32 output feature tiles
NPAIR = NUM_HEADS // 2         # 16 head pairs
NB = 2                         # 512-wide matmul chunks (PSUM bank limit)
NW = TOK // NB
# fp8 build measured at rel-err 2.5e-3 on the harness inputs (gate=0.786),
# ~8x inside the 2e-2 gate, and ~1.2x faster than the bf16 build. Use it
# for any gate; env override kept for A/B testing.
FP8_GATE_MAX = float(os.environ.get("KFP8_GATE", "10.0"))

_PATCHED = False


def dedupe_ldweights(nc):
    """Remove InstLdweights that reload the exact weights already resident
    in the PE array (same physical AP + perf_mode, no intervening weight
    change, no sync side effects). The following matmuls then reuse the
    loaded stationary operand; verified correct on hw (microbench)."""
    removed = 0
    skippable = ("InstMatmult", "InstNoOp", "InstEventSemaphore")
    for f in nc.m.functions:
        for b in f.blocks:
            insts = b.instructions
            last_sig = None
            to_remove = []
            for i in insts:
                tn = type(i).__name__
                if tn == "InstLdweights":
                    si = i.sync_info
                    clean = si is None or (not si.on_wait and not si.on_update)
                    sig = (repr(i.ins[0]), str(i.perf_mode), bool(i.is_transpose))
                    if sig == last_sig and clean:
                        to_remove.append(i)
                        continue
                    last_sig = sig
                elif tn in skippable:
                    pass
                elif getattr(i, "engine", None) == mybir.EngineType.PE:
                    last_sig = None
            for i in to_remove:
                insts.remove(i)
            removed += len(to_remove)
    return removed


def _patch_tile():
    """TRN2 instructions have one hw wait slot; walrus rejects multi-wait
    matmuls and the kernel-tail drain. Hoist extra waits onto NoOps."""
    global _PATCHED
    if _PATCHED:
        return
    _PATCHED = True
    import concourse.tile as _tile
    from concourse.vector_clock import ScopedClock

    _orig_commit = _tile.TileContext._commit_instruction

    def _patched_commit(self, inst, lazy_reg_writes=True):
        si = getattr(inst, "sync_info", None)
        if (
            si is not None
            and si.on_wait
            and len(si.on_wait) > 1
            and inst.engine != mybir.EngineType.Unassigned
            and not isinstance(inst, mybir.InstNoOp)
        ):
            waits = list(si.on_wait)
            for w in waits[:-1]:
                nop = mybir.InstNoOp(
                    name=self.nc.get_next_instruction_name(),
                    ins=[], outs=[], bass_nofuse=True,
                )
                nop.engine = inst.engine
                nop.sync_info = mybir.SyncInfo(on_wait=[w], on_update=[])
                _orig_commit(self, nop, lazy_reg_writes=False)
            inst.sync_info = mybir.SyncInfo(
                on_wait=waits[-1:], on_update=list(si.on_update or [])
            )
        return _orig_commit(self, inst, lazy_reg_writes=lazy_reg_writes)

    def _patched_drain_and_barrier(self, tick_clock, wait_clock):
        nc = self.nc
        carrier = nc.sync.nop(nofuse=True)
        wait_clock.add_sem_waits(
            carrier.ins, ScopedClock({None: tick_clock.global_clock})
        )
        si = carrier.ins.sync_info
        waits = list(si.on_wait) if si and si.on_wait else []
        if len(waits) > 1:
            carrier.ins.sync_info = mybir.SyncInfo(
                on_wait=waits[:1], on_update=list(si.on_update or [])
            )
            for w in waits[1:]:
                extra = nc.sync.nop(nofuse=True)
                extra.ins.sync_info = mybir.SyncInfo(on_wait=[w], on_update=[])
        nc.sync.drain()
        nc.all_engine_barrier()
        assert self.sems is not None
        popped = nc._tile_sem_poison_stack.pop()
        assert popped is self._sem_poison
        nc.clear_and_free_semaphores(list(self.sems.allocated().values()))
        nc.all_engine_barrier()

    _tile.TileContext._commit_instruction = _patched_commit
    _tile.TileContext._drain_and_barrier = _patched_drain_and_barrier


def build_fp8():
    _patch_tile()
    from contextlib import ExitStack

    nc = bass.Bass(target_bir_lowering=False)

    xq8 = nc.declare_dram_parameter("xq8", [KT // 2, P, 2, TOK], F8, isOutput=False)
    qw8 = nc.declare_dram_parameter("qw8", [MT, 4, P, 4, 2, P], F8, isOutput=False)
    vw8 = nc.declare_dram_parameter("vw8", [MT, 4, P, 2, 2, P], F8, isOutput=False)
    katt = nc.declare_dram_parameter("katt", [P, NUM_HEADS * L], BF16, isOutput=False)
    blkones = nc.declare_dram_parameter("blkones", [P, P], BF16, isOutput=False)
    cosT = nc.declare_dram_parameter("cosT", [HD, TOK], BF16, isOutput=False)
    srotT = nc.declare_dram_parameter("srotT", [HD, TOK], BF16, isOutput=False)
    qb2 = nc.declare_dram_parameter("qb2", [P, MT], F32, isOutput=False)
    qbr = nc.declare_dram_parameter("qbr", [P, MT], F32, isOutput=False)
    baseT = nc.declare_dram_parameter("baseT", [H, TOK], BF16, isOutput=False)
    dsc = nc.declare_dram_parameter("dsc", [P, 1], F32, isOutput=False)
    outT = nc.declare_dram_parameter("outT", [H, TOK], BF16, isOutput=True)

    LN16 = float(np.log(16.0))

    with tile.TileContext(nc) as tc:
        es = ExitStack()
        persist = es.enter_context(tc.tile_pool(name="persist", bufs=1))

        ph_es = ExitStack()
        psQ = ph_es.enter_context(tc.tile_pool(name="psQ", bufs=4, space="PSUM"))
        psS = ph_es.enter_context(tc.tile_pool(name="psS", bufs=2, space="PSUM"))
        wpool = ph_es.enter_context(tc.tile_pool(name="wpool", bufs=20))
        probs_pool = ph_es.enter_context(tc.tile_pool(name="probs", bufs=1))
        # dedicated pool for the first phase-2 weight tiles (prefetched
        # mid-phase-1); must be created before the xt-scope pools so pool
        # releases stay LIFO
        vpool = ph_es.enter_context(tc.tile_pool(name="vpool", bufs=12))
        xt_es = ExitStack()
        xt_pool = xt_es.enter_context(tc.tile_pool(name="xt", bufs=1))
        rpool = xt_es.enter_context(tc.tile_pool(name="rpool", bufs=2))
        tpool = xt_es.enter_context(tc.tile_pool(name="tpool", bufs=2))

        # ---- urgent streams, interleaved by first-use: the first matmuls
        # need xq d0-d3 + the first head's 4 weight tiles, so those lead;
        # the rest follow. gpsimd DMAs avoided everywhere: software DGE
        # pulls in a library load that delays kernel start by ~3us.
        xq_tiles = [None] * (KT // 2)
        qw_pref = {}

        def _load_xq(d, eng):
            t = xt_pool.tile([P, 2, TOK], F8, name=f"xq_{d}", tag=f"xq_{d}")
            eng.dma_start(out=t[:], in_=xq8[d])
            xq_tiles[d] = t

        def _load_qw(h, c, eng):
            wb = wpool.tile([P, 4, 2, P], F8, name=f"qw_{h}_{c}", tag="wq")
            eng.dma_start(out=wb[:], in_=qw8[h, c])
            qw_pref[(h, c)] = wb

        # heads are processed in rotated order HSEQ (see below): first two
        # processed heads are 4 and 5
        _load_xq(0, nc.sync)
        _load_xq(1, nc.scalar)
        _load_qw(4, 0, nc.sync)
        _load_qw(4, 1, nc.scalar)
        _load_xq(2, nc.sync)
        _load_xq(3, nc.scalar)
        _load_qw(4, 2, nc.sync)
        _load_qw(4, 3, nc.scalar)
        for d in range(4, KT // 2):
            _load_xq(d, (nc.sync, nc.scalar)[d % 2])
        for c in range(4):
            _load_qw(5, c, (nc.sync, nc.scalar)[c % 2])

        cos_sb = persist.tile([HD, TOK], BF16, name="cos_sb")
        nc.sync.dma_start(out=cos_sb[:], in_=cosT[:])
        srot_sb = persist.tile([HD, TOK], BF16, name="srot_sb")
        nc.scalar.dma_start(out=srot_sb[:], in_=srotT[:])
        katt_sb = persist.tile([P, NUM_HEADS * L], BF16, name="katt_sb")
        nc.sync.dma_start(out=katt_sb[:], in_=katt[:])
        blk_sb = persist.tile([P, P], BF16, name="blk_sb")
        nc.sync.dma_start(out=blk_sb[:], in_=blkones[:])
        qb2_sb = persist.tile([P, MT], F32, name="qb2_sb")
        nc.scalar.dma_start(out=qb2_sb[:], in_=qb2[:])
        qbr_sb = persist.tile([P, MT], F32, name="qbr_sb")
        nc.scalar.dma_start(out=qbr_sb[:], in_=qbr[:])
        dsc_sb = persist.tile([P, 1], F32, name="dsc_sb")
        nc.sync.dma_start(out=dsc_sb[:], in_=dsc[:])
        ln16_sb = persist.tile([P, 1], F32, name="ln16_sb")
        nc.vector.memset(ln16_sb[:], LN16)
        s16_sb = persist.tile([P, 1], F32, name="s16_sb")
        nc.vector.memset(s16_sb[:], 16.0)

        probs_quads = [
            probs_pool.tile([P, 2, TOK], F8, name=f"probs_{q}", tag=f"probs_{q}")
            for q in range(NPAIR // 2)
        ]

        sc_pair = [None]
        expt_tiles = {}
        den_tiles = {}

        def emit_q(h, pqs=None):
            # per-j [P, NW] psum tiles on a 4-deep ring: each half frees as
            # soon as its own drain copy runs, so the next q block never
            # waits on the slower half's reader
            if pqs is None:
                pqs = [psQ.tile([P, NW], F32, name=f"pq_{h}_{j}", tag="psQ",
                                bufs=4) for j in range(NB)]
            wbs = []
            for c in range(4):
                if (h, c) in qw_pref:
                    wb = qw_pref[(h, c)]
                else:
                    wb = wpool.tile([P, 4, 2, P], F8, name=f"qw_{h}_{c}", tag="wq")
                    eng = (nc.sync, nc.scalar)[(h * 4 + c) % 2]
                    eng.dma_start(out=wb[:], in_=qw8[h, c])
                wbs.append(wb)
            # j innermost: both 512-token halves stream against one weight
            # load (the dedupe pass drops the second LDWEIGHTS)
            for c in range(4):
                for dd in range(4):
                    d = c * 4 + dd
                    for j in range(NB):
                        nc.tensor.matmul(
                            out=pqs[j][:],
                            lhsT=wbs[c][:, dd, :, :],
                            rhs=xq_tiles[d][:, :, j * NW:(j + 1) * NW],
                            start=(d == 0), stop=(d == KT // 2 - 1),
                            perf_mode=mybir.MatmulPerfMode.DoubleRow,
                        )
            return pqs

        def emit_rope(h, pqs):
            q_sb = rpool.tile([P, TOK], BF16, name=f"q_sb_{h}", tag="q_sb")
            # split the PSUM drain across two engines so each half's buffer
            # frees independently
            nc.scalar.copy(out=q_sb[:, :NW], in_=pqs[0][:])
            nc.vector.tensor_copy(out=q_sb[:, NW:], in_=pqs[1][:])
            rot = rpool.tile([P, TOK], BF16, name=f"rot_{h}", tag="rot")
            nc.scalar.dma_start(out=rot[0:64, :], in_=q_sb[64:128, :])
            nc.sync.dma_start(out=rot[64:128, :], in_=q_sb[0:64, :])
            t2 = rpool.tile([P, TOK], BF16, name=f"t2_{h}", tag="t2")
            nc.vector.scalar_tensor_tensor(
                out=t2[:], in0=q_sb[:], scalar=qb2_sb[:, h:h + 1],
                in1=cos_sb[:], op0=mybir.AluOpType.add,
                op1=mybir.AluOpType.mult)
            t1 = rpool.tile([P, TOK], BF16, name=f"t1_{h}", tag="t1")
            nc.vector.scalar_tensor_tensor(
                out=t1[:], in0=rot[:], scalar=qbr_sb[:, h:h + 1],
                in1=srot_sb[:], op0=mybir.AluOpType.add,
                op1=mybir.AluOpType.mult)
            qr = rpool.tile([P, TOK], BF16, name=f"qr_{h}", tag="qr", bufs=3)
            nc.vector.tensor_add(out=qr[:], in0=t2[:], in1=t1[:])
            return qr

        qr_even = [None]

        def emit_sc(h, qr):
            # defer the even head; emit both heads' score matmuls together
            # so the two 64-partition outputs run CONCURRENTLY in separate
            # PE column groups (tile_position derives from out partition
            # offset) — 4 MMs in ~2 MM times instead of 4
            hh = h % 2
            p = h // 2
            if hh == 0:
                qr_even[0] = qr
                return
            sc = psS.tile([P, TOK], F32, name=f"sc_{p}", tag="psS")
            sc_pair[0] = sc
            for j in range(NB):
                for hh2, qrx in ((0, qr_even[0]), (1, qr)):
                    hx = 2 * p + hh2
                    nc.tensor.matmul(
                        out=sc[hh2 * 64:(hh2 + 1) * 64, j * NW:(j + 1) * NW],
                        lhsT=katt_sb[:, hx * L:(hx + 1) * L],
                        rhs=qrx[:, j * NW:(j + 1) * NW],
                        start=True, stop=True,
                    )
            expt = tpool.tile([P, TOK], BF16, name=f"expt_{p}",
                              tag="expt", bufs=2)
            nc.scalar.activation(expt[:], sc[:],
                                 mybir.ActivationFunctionType.Exp)
            expt_tiles[p] = (expt, sc)

        def emit_den(p, fast_tail=False):
            expt, scp = expt_tiles.pop(p)
            dens = [psQ.tile([P, NW], F32, name=f"den_{p}_{j}", tag="psQ",
                             bufs=4) for j in range(NB)]
            for j in range(NB):
                nc.tensor.matmul(
                    out=dens[j][:],
                    lhsT=blk_sb[:],
                    rhs=expt[:, j * NW:(j + 1) * NW],
                    start=True, stop=True,
                )
            # NOTE: a vector.reciprocal softmax tail was tried here and is
            # ~3.3us per 512-col half on DVE — far slower than this chain.
            lnden = tpool.tile([P, TOK], F32, name=f"lnden_{p}",
                               tag="lnden", bufs=2)
            d2 = tpool.tile([P, TOK], F32, name=f"d2_{p}", tag="d2", bufs=2)
            if fast_tail:
                # last pair: fully j-split ln->sub->exp chain so the j0
                # half of the quad is ready ~1.2us earlier for phase 2
                for j in range(NB):
                    sl = slice(j * NW, (j + 1) * NW)
                    nc.scalar.activation(lnden[:, sl], dens[j][:],
                                         mybir.ActivationFunctionType.Ln)
                    nc.vector.tensor_sub(out=d2[:, sl], in0=scp[:, sl],
                                         in1=lnden[:, sl])
                    nc.scalar.activation(
                        probs_quads[p // 2][:, p % 2, sl], d2[:, sl],
                        mybir.ActivationFunctionType.Exp,
                        bias=ln16_sb[:, 0:1])
                return
            for j in range(NB):
                nc.scalar.activation(lnden[:, j * NW:(j + 1) * NW],
                                     dens[j][:],
                                     mybir.ActivationFunctionType.Ln)
            nc.vector.tensor_sub(out=d2[:], in0=scp[:], in1=lnden[:])
            nc.scalar.activation(
                probs_quads[p // 2][:, p % 2, :], d2[:],
                mybir.ActivationFunctionType.Exp, bias=ln16_sb[:, 0:1])

        # Heads processed in rotated order: heads 0..3 (probs quad 0) come
        # LAST, and phase 2 contracts quad 0 last, so the final heads'
        # softmax chain (a ~5us scalar/vector serial tail) hides behind
        # phase-2 matmuls on quads 1..7.
        HSEQ = list(range(4, NUM_HEADS)) + [0, 1, 2, 3]
        PSEQ = [h // 2 for h in HSEQ[::2]]  # pair completion order

        # first phase-2 weight tiles prefetch mid-phase-1 from vpool so
        # their DMAs are not gated on the q-weight ring freeing (that
        # wait was a 4.4us PE stall at the phase boundary)
        vwo_pref = {}
        m0_pos = [None]
        m0_wbs = [None]

        # NOTE: a HAM warm-up (48 zero matmuls during the initial DMA wait)
        # was tried here and regressed exec by ~70us — the repeated
        # single-MM start/stop groups serialize pathologically. Don't.
        pq_of = {}
        for step in range(NUM_HEADS):
            pq_of[HSEQ[step]] = emit_q(HSEQ[step])
            if step == 26:
                for m in range(3):
                    for c in range(4):
                        wb = vpool.tile([P, 2, 2, P], F8,
                                        name=f"vw_{m}_{c}", tag="wv0")
                        eng = (nc.sync, nc.scalar)[(m * 4 + c) % 2]
                        eng.dma_start(out=wb[:], in_=vw8[m, c])
                        vwo_pref[(m, c)] = wb
            if step >= 1:
                h = HSEQ[step - 1]
                qr = emit_rope(h, pq_of.pop(h))
                emit_sc(h, qr)
            if step >= 4 and (step % 2) == 0:
                emit_den(PSEQ[(step - 4) // 2])

        # ---- explicit phase-1 tail, ordered to keep the PE fed while the
        # last pairs' softmax drains on scalar/vector:
        #   rope(3) queued first (vector chain overlaps the PE work below)
        #   den(p0), then m0's quads 1..7 (3.2us of PE work),
        #   sc pair p1, then den(p1) via the vector-only recip path.
        h_last = HSEQ[-1]
        qr_last = emit_rope(h_last, pq_of.pop(h_last))
        emit_den(PSEQ[14])
        pos = [psQ.tile([P, NW], F32, name=f"po_0_{j}", tag="psQ",
                        bufs=4) for j in range(NB)]
        wbs = [vwo_pref[(0, c)] for c in range(4)]
        for q in range(1, NPAIR // 2):
            for j in range(NB):
                nc.tensor.matmul(
                    out=pos[j][:],
                    lhsT=wbs[q // 2][:, q % 2, :, :],
                    rhs=probs_quads[q][:, :, j * NW:(j + 1) * NW],
                    start=(q == 1), stop=False,
                    perf_mode=mybir.MatmulPerfMode.DoubleRow,
                )
        m0_pos[0] = pos
        m0_wbs[0] = wbs
        emit_sc(h_last, qr_last)
        emit_den(PSEQ[15])

        xt_es.close()

        # ---- phase 2: out = probs @ VWO (DR) * dsc + base ----
        p2_es = ExitStack()
        bpool = p2_es.enter_context(tc.tile_pool(name="bpool", bufs=4))

        # quad 0 contracts last (its probs complete last under HSEQ)
        QSEQ = list(range(1, NPAIR // 2)) + [0]
        for m in range(MT):
            if m == 0:
                # quads 1..7 already emitted at the phase boundary
                pos, wbs = m0_pos[0], m0_wbs[0]
                qlist = [0]
            else:
                pos = [psQ.tile([P, NW], F32, name=f"po_{m}_{j}", tag="psQ",
                                bufs=4) for j in range(NB)]
                wbs = []
                for c in range(4):
                    if (m, c) in vwo_pref:
                        wb = vwo_pref[(m, c)]
                    else:
                        wb = wpool.tile([P, 2, 2, P], F8, name=f"vw_{m}_{c}",
                                        tag="wv")
                        eng = (nc.sync, nc.scalar)[(m * 4 + c) % 2]
                        eng.dma_start(out=wb[:], in_=vw8[m, c])
                    wbs.append(wb)
                qlist = QSEQ
            bt = bpool.tile([P, TOK], BF16, name=f"bt_{m}", tag="bt")
            # alternate queues: putting all 8MB of base tiles on the scalar
            # queue (which also carries half the vw weights + stores) made
            # weight halves arrive late -> scattered phase-2 PE gaps
            (nc.sync, nc.scalar)[(m + 1) % 2].dma_start(
                out=bt[:], in_=baseT[m * P:(m + 1) * P, :])
            for q in qlist:
                for j in range(NB):
                    nc.tensor.matmul(
                        out=pos[j][:],
                        lhsT=wbs[q // 2][:, q % 2, :, :],
                        rhs=probs_quads[q][:, :, j * NW:(j + 1) * NW],
                        start=(m != 0 and q == QSEQ[0]), stop=(q == 0),
                        perf_mode=mybir.MatmulPerfMode.DoubleRow,
                    )
            os_ = bpool.tile([P, TOK], BF16, name=f"os_{m}", tag="os")
            for j in range(NB):
                nc.vector.scalar_tensor_tensor(
                    out=os_[:, j * NW:(j + 1) * NW], in0=pos[j][:],
                    scalar=dsc_sb[:, 0:1],
                    in1=bt[:, j * NW:(j + 1) * NW], op0=mybir.AluOpType.mult,
                    op1=mybir.AluOpType.add)
                (nc.sync, nc.scalar)[(2 * m + j) % 2].dma_start(
                    out=outT[m * P:(m + 1) * P, j * NW:(j + 1) * NW],
                    in_=os_[:, j * NW:(j + 1) * NW])

        p2_es.close()
        ph_es.close()
        es.close()
    dedupe_ldweights(nc)
    return nc


def build_bf16():
    _patch_tile()
    from contextlib import ExitStack

    nc = bass.Bass(target_bir_lowering=False)

    xT = nc.declare_dram_parameter("xT", [H, TOK], BF16, isOutput=False)
    qwB = nc.declare_dram_parameter("qwB", [MT, 8, P, 4, P], BF16, isOutput=False)
    vwoB = nc.declare_dram_parameter("vwoB", [MT, 4, P, 4, P], BF16, isOutput=False)
    katt = nc.declare_dram_parameter("katt", [P, NUM_HEADS * L], BF16, isOutput=False)
    blkones = nc.declare_dram_parameter("blkones", [P, P], BF16, isOutput=False)
    cosT = nc.declare_dram_parameter("cosT", [HD, TOK], F32, isOutput=False)
    srotT = nc.declare_dram_parameter("srotT", [HD, TOK], F32, isOutput=False)
    qb2 = nc.declare_dram_parameter("qb2", [P, MT], F32, isOutput=False)
    qbr = nc.declare_dram_parameter("qbr", [P, MT], F32, isOutput=False)
    baseT = nc.declare_dram_parameter("baseT", [H, TOK], F32, isOutput=False)
    outT = nc.declare_dram_parameter("outT", [H, TOK], F32, isOutput=True)

    with tile.TileContext(nc) as tc:
        es = ExitStack()
        persist = es.enter_context(tc.tile_pool(name="persist", bufs=1))

        ph_es = ExitStack()
        psQ = ph_es.enter_context(tc.tile_pool(name="psQ", bufs=2, space="PSUM"))
        psS = ph_es.enter_context(tc.tile_pool(name="psS", bufs=1, space="PSUM"))
        psD = ph_es.enter_context(tc.tile_pool(name="psD", bufs=1, space="PSUM"))
        wpool = ph_es.enter_context(tc.tile_pool(name="wpool", bufs=20))
        probs_pool = ph_es.enter_context(tc.tile_pool(name="probs", bufs=1))
        xt_es = ExitStack()
        xt_pool = xt_es.enter_context(tc.tile_pool(name="xt", bufs=1))
        rpool = xt_es.enter_context(tc.tile_pool(name="rpool", bufs=2))
        tpool = xt_es.enter_context(tc.tile_pool(name="tpool", bufs=2))

        qw_pref = {}
        for h in range(2):
            for kg in range(8):
                wb = wpool.tile([P, 4, P], BF16, name=f"qw_{h}_{kg}", tag="wq")
                eng = (nc.sync, nc.scalar, nc.gpsimd)[kg % 3]
                eng.dma_start(out=wb[:], in_=qwB[h, kg])
                qw_pref[(h, kg)] = wb
        xt_tiles = []
        for k in range(KT):
            t = xt_pool.tile([P, TOK], BF16, name=f"xt_{k}", tag=f"xt_{k}")
            eng = (nc.sync, nc.scalar, nc.gpsimd)[k % 3]
            eng.dma_start(out=t[:], in_=xT[k * P:(k + 1) * P, :])
            xt_tiles.append(t)

        cos_sb = persist.tile([HD, TOK], F32, name="cos_sb")
        nc.gpsimd.dma_start(out=cos_sb[:], in_=cosT[:])
        srot_sb = persist.tile([HD, TOK], F32, name="srot_sb")
        nc.scalar.dma_start(out=srot_sb[:], in_=srotT[:])
        katt_sb = persist.tile([P, NUM_HEADS * L], BF16, name="katt_sb")
        nc.sync.dma_start(out=katt_sb[:], in_=katt[:])
        blk_sb = persist.tile([P, P], BF16, name="blk_sb")
        nc.sync.dma_start(out=blk_sb[:], in_=blkones[:])
        qb2_sb = persist.tile([P, MT], F32, name="qb2_sb")
        nc.gpsimd.dma_start(out=qb2_sb[:], in_=qb2[:])
        qbr_sb = persist.tile([P, MT], F32, name="qbr_sb")
        nc.scalar.dma_start(out=qbr_sb[:], in_=qbr[:])

        probs_tiles = [
            probs_pool.tile([P, TOK], BF16, name=f"probs_{p}", tag=f"probs_{p}")
            for p in range(NPAIR)
        ]

        sc_pair = [None]
        expt_tiles = {}

        def emit_q(h):
            pq = psQ.tile([P, TOK], F32, name=f"pq_{h}", tag="psQ")
            wbs = []
            for kg in range(8):
                if (h, kg) in qw_pref:
                    wb = qw_pref[(h, kg)]
                else:
                    wb = wpool.tile([P, 4, P], BF16, name=f"qw_{h}_{kg}", tag="wq")
                    eng = (nc.sync, nc.scalar, nc.gpsimd)[(h * 8 + kg) % 3]
                    eng.dma_start(out=wb[:], in_=qwB[h, kg])
                wbs.append(wb)
            for j in range(NB):
                for kg in range(8):
                    for kk in range(4):
                        k = kg * 4 + kk
                        nc.tensor.matmul(
                            out=pq[:, j * NW:(j + 1) * NW],
                            lhsT=wbs[kg][:, kk, :],
                            rhs=xt_tiles[k][:, j * NW:(j + 1) * NW],
                            start=(k == 0), stop=(k == KT - 1),
                        )
            return pq

        def emit_rope(h, pq):
            q_sb = rpool.tile([P, TOK], BF16, name=f"q_sb_{h}", tag="q_sb")
            nc.scalar.copy(out=q_sb[:], in_=pq[:])
            rot = rpool.tile([P, TOK], BF16, name=f"rot_{h}", tag="rot")
            nc.scalar.dma_start(out=rot[0:64, :], in_=q_sb[64:128, :])
            nc.sync.dma_start(out=rot[64:128, :], in_=q_sb[0:64, :])
            t2 = rpool.tile([P, TOK], F32, name=f"t2_{h}", tag="t2")
            nc.vector.scalar_tensor_tensor(
                out=t2[:], in0=q_sb[:], scalar=qb2_sb[:, h:h + 1],
                in1=cos_sb[:], op0=mybir.AluOpType.add,
                op1=mybir.AluOpType.mult)
            t1 = rpool.tile([P, TOK], F32, name=f"t1_{h}", tag="t1")
            nc.vector.scalar_tensor_tensor(
                out=t1[:], in0=rot[:], scalar=qbr_sb[:, h:h + 1],
                in1=srot_sb[:], op0=mybir.AluOpType.add,
                op1=mybir.AluOpType.mult)
            qr = rpool.tile([P, TOK], BF16, name=f"qr_{h}", tag="qr", bufs=3)
            nc.vector.tensor_add(out=qr[:], in0=t2[:], in1=t1[:])
            return qr

        def emit_sc(h, qr):
            hh = h % 2
            p = h // 2
            if hh == 0:
                sc_pair[0] = psS.tile([P, TOK], F32, name=f"sc_{p}", tag="psS")
            for j in range(NB):
                nc.tensor.matmul(
                    out=sc_pair[0][hh * 64:(hh + 1) * 64, j * NW:(j + 1) * NW],
                    lhsT=katt_sb[:, h * L:(h + 1) * L],
                    rhs=qr[:, j * NW:(j + 1) * NW],
                    start=True, stop=True,
                )
            if hh == 1:
                expt = tpool.tile([P, TOK], BF16, name=f"expt_{p}",
                                  tag="expt", bufs=2)
                nc.scalar.activation(expt[:], sc_pair[0][:],
                                     mybir.ActivationFunctionType.Exp)
                expt_tiles[p] = expt

        def emit_den(p):
            expt = expt_tiles.pop(p)
            den = psD.tile([P, TOK], F32, name=f"den_{p}", tag="psD")
            for j in range(NB):
                nc.tensor.matmul(
                    out=den[:, j * NW:(j + 1) * NW],
                    lhsT=blk_sb[:],
                    rhs=expt[:, j * NW:(j + 1) * NW],
                    start=True, stop=True,
                )
            recip = tpool.tile([P, TOK], F32, name=f"recip_{p}",
                               tag="recip", bufs=2)
            nc.vector.reciprocal(out=recip[:], in_=den[:])
            nc.vector.tensor_mul(out=probs_tiles[p][:], in0=expt[:],
                                 in1=recip[:])

        pq_of = {}
        for step in range(NUM_HEADS + 2):
            if step < NUM_HEADS:
                pq_of[step] = emit_q(step)
            if step >= 3 and (step % 2) == 1:
                emit_den((step - 3) // 2)
            if 1 <= step <= NUM_HEADS:
                h = step - 1
                qr = emit_rope(h, pq_of.pop(h))
                emit_sc(h, qr)

        vwo_pref = {}
        for m in range(2):
            for g in range(4):
                wb = wpool.tile([P, 4, P], BF16, name=f"vw_{m}_{g}", tag="wq")
                eng = (nc.sync, nc.scalar, nc.gpsimd)[(m * 4 + g) % 3]
                eng.dma_start(out=wb[:], in_=vwoB[m, g])
                vwo_pref[(m, g)] = wb

        xt_es.close()

        p2_es = ExitStack()
        bpool = p2_es.enter_context(tc.tile_pool(name="bpool", bufs=3))

        for m in range(MT):
            po = psQ.tile([P, TOK], F32, name=f"po_{m}", tag="psQ")
            wbs = []
            for g in range(4):
                if (m, g) in vwo_pref:
                    wb = vwo_pref[(m, g)]
                else:
                    wb = wpool.tile([P, 4, P], BF16, name=f"vw_{m}_{g}", tag="wq")
                    eng = (nc.sync, nc.scalar, nc.gpsimd)[(m * 4 + g) % 3]
                    eng.dma_start(out=wb[:], in_=vwoB[m, g])
                wbs.append(wb)
            for j in range(NB):
                for g in range(4):
                    for i in range(4):
                        pr = g * 4 + i
                        nc.tensor.matmul(
                            out=po[:, j * NW:(j + 1) * NW],
                            lhsT=wbs[g][:, i, :],
                            rhs=probs_tiles[pr][:, j * NW:(j + 1) * NW],
                            start=(pr == 0), stop=(pr == NPAIR - 1),
                        )
            bt = bpool.tile([P, TOK], F32, name=f"bt_{m}", tag="bt")
            nc.scalar.dma_start(out=bt[:], in_=baseT[m * P:(m + 1) * P, :])
            os_ = bpool.tile([P, TOK], F32, name=f"os_{m}", tag="os")
            nc.vector.tensor_add(out=os_[:], in0=po[:], in1=bt[:])
            (nc.sync, nc.scalar)[m % 2].dma_start(
                out=outT[m * P:(m + 1) * P, :], in_=os_[:])

        p2_es.close()
        ph_es.close()
        es.close()
    return nc


_NC_CACHE = {}


def _pow2_scale(absmax, target=224.0):
    return 2.0 ** math.floor(math.log2(target / max(absmax, 1e-30)))


def kernel(hidden_states, position_ids, base_output, cos, sin,
           q_w, k_w, v_w, o_w,
           q_scale, k_scale, v_scale, o_scale,
           q_bias, k_bias, v_bias, o_bias,
           adaption_prompt, adaption_gate):
    import ml_dtypes

    hidden_states = np.asarray(hidden_states, dtype=np.float32)
    base_output = np.asarray(base_output, dtype=np.float32)
    pos = np.asarray(position_ids).reshape(-1).astype(np.int64)
    cos = np.asarray(cos, dtype=np.float32)
    sin = np.asarray(sin, dtype=np.float32)

    X = hidden_states.reshape(B * S, H)
    BASE = (base_output.reshape(B * S, H)
            + np.asarray(o_bias, dtype=np.float32)[None, :])
    cosg = cos[0, 0][pos]                     # [B*S, HD]
    sing = sin[0, 0][pos]
    sr = sing.copy()
    sr[:, :HD // 2] *= -1.0                   # sign for rotate_half product

    inv = 1.0 / np.sqrt(HD)
    gate = float(np.asarray(adaption_gate).reshape(-1)[0])
    q_wT = (np.asarray(q_w, dtype=np.float64)
            * np.asarray(q_scale, dtype=np.float64)[None, :]).T
    k_wT = (np.asarray(k_w, dtype=np.float64)
            * np.asarray(k_scale, dtype=np.float64)[None, :]).T
    v_wT = (np.asarray(v_w, dtype=np.float64)
            * np.asarray(v_scale, dtype=np.float64)[None, :]).T
    o_wT = (np.asarray(o_w, dtype=np.float64)
            * np.asarray(o_scale, dtype=np.float64)[None, :]).T
    pr = np.asarray(adaption_prompt, dtype=np.float64)[0]      # [L, H]

    # weight-only folds (host, f64): adapter K/V and per-head V_h @ Wo_h
    ak = (pr @ k_wT + np.asarray(k_bias, dtype=np.float64)[None, :]) * inv
    av = (pr @ v_wT + np.asarray(v_bias, dtype=np.float64)[None, :]) * gate
    VWO = np.empty((NUM_HEADS * L, H))
    for h in range(NUM_HEADS):
        VWO[h * L:(h + 1) * L, :] = (
            av[:, h * HD:(h + 1) * HD] @ o_wT[h * HD:(h + 1) * HD, :])
    katt_ = np.empty((P, NUM_HEADS * L), dtype=np.float32)
    for h in range(NUM_HEADS):
        katt_[:, h * L:(h + 1) * L] = ak[:, h * HD:(h + 1) * HD].T
    blk = np.zeros((P, P), dtype=np.float32)
    blk[:64, :64] = 1.0
    blk[64:, 64:] = 1.0

    use_fp8 = gate < FP8_GATE_MAX
    variant = "fp8" if use_fp8 else "bf16"
    qb_base = np.asarray(q_bias, dtype=np.float32).reshape(MT, P).T

    if use_fp8:
        sx = 32.0
        sw = _pow2_scale(np.abs(q_wT).max())
        sv = _pow2_scale(np.abs(VWO).max())
        # descale 1/(sx*sw) folded into cos/srot; q bias pre-scaled up
        qb2_ = np.ascontiguousarray(qb_base * (sx * sw)).astype(np.float32)
        qbr_ = np.ascontiguousarray(np.roll(qb2_, -64, axis=0))
        cos_h = (cosg / (sx * sw)).astype(np.float32)
        sr_h = (sr / (sx * sw)).astype(np.float32)
        qw8_ = np.ascontiguousarray(
            (q_wT * sw).reshape(4, 4, 2, P, MT, P).transpose(4, 0, 3, 1, 2, 5)
        ).astype(ml_dtypes.float8_e4m3)
        vw8_ = np.ascontiguousarray(
            (VWO * sv).reshape(4, 2, 2, P, MT, P).transpose(4, 0, 3, 1, 2, 5)
        ).astype(ml_dtypes.float8_e4m3)
        dsc_ = np.full((P, 1), 1.0 / (16.0 * sv), dtype=np.float32)
    else:
        qb2_ = np.ascontiguousarray(qb_base)
        qbr_ = np.ascontiguousarray(np.roll(qb2_, -64, axis=0))
        cos_h = cosg.astype(np.float32)
        sr_h = sr.astype(np.float32)
        qwB_ = np.ascontiguousarray(
            q_wT.reshape(8, 4, P, MT, P).transpose(3, 0, 2, 1, 4)
        ).astype(ml_dtypes.bfloat16)
        vwoB_ = np.ascontiguousarray(
            VWO.reshape(4, 4, P, MT, P).transpose(3, 0, 2, 1, 4)
        ).astype(ml_dtypes.bfloat16)

    katt_b = katt_.astype(ml_dtypes.bfloat16)
    blk_b = blk.astype(ml_dtypes.bfloat16)

    in_maps = []
    for c in range(NC):
        sl = slice(c * TOK, (c + 1) * TOK)
        m = {
            "katt": katt_b,
            "blkones": blk_b,
            "qb2": qb2_,
            "qbr": qbr_,
        }
        if use_fp8:
            xc = np.ascontiguousarray(
                (X[sl].T * sx).reshape(KT // 2, 2, P, TOK).transpose(0, 2, 1, 3)
            ).astype(ml_dtypes.float8_e4m3)
            m.update({
                "xq8": xc,
                "qw8": qw8_,
                "vw8": vw8_,
                "cosT": np.ascontiguousarray(cos_h[sl].T).astype(ml_dtypes.bfloat16),
                "srotT": np.ascontiguousarray(sr_h[sl].T).astype(ml_dtypes.bfloat16),
                "baseT": np.ascontiguousarray(BASE[sl].T).astype(ml_dtypes.bfloat16),
                "dsc": dsc_,
            })
        else:
            m.update({
                "xT": np.ascontiguousarray(X[sl].T).astype(ml_dtypes.bfloat16),
                "qwB": qwB_,
                "vwoB": vwoB_,
                "cosT": np.ascontiguousarray(cos_h[sl].T),
                "srotT": np.ascontiguousarray(sr_h[sl].T),
                "baseT": np.ascontiguousarray(BASE[sl].T),
            })
        in_maps.append(m)

    if variant not in _NC_CACHE:
        _NC_CACHE[variant] = build_fp8() if use_fp8 else build_bf16()
    nc = _NC_CACHE[variant]

    trace = bool(os.environ.get("KERNEL_TRACE"))
    res = run_bass_kernel_spmd(nc, in_maps, core_ids=list(range(NC)),
                               trace=trace)
    if trace and res.exec_time_ns is not None:
        print(f"HW exec time: {res.exec_time_ns} ns")

    out = np.empty((B * S, H), dtype=np.float32)
    for c in range(NC):
        out[c * TOK:(c + 1) * TOK, :] = res.results[c]["outT"].T
    return out.reshape(B, S, H)

